# revision 31
# baseline (speedup 1.0000x reference)
"""DiT block kernel for 8 Trainium2 NeuronCores.

Sharding: data-parallel over batch (B=8 -> one batch element per core).

Vs the original bf16 baseline:
  - scores / ada / fc1 / fc2 use 512-wide moving operands (baseline used
    256) — wide streams hide the PE weight-load latency.
  - q/k/attn stored fp8(e4m3): halves SBUF + eviction cost; scores and
    attn@V run with fp8 moving operands at bf16 rate; proj streams fp8
    per-column-quantized weights (dequant folded into the gate tile).
  - LN: token-major norm -> PE transpose -> modulate fused into ONE
    tensor_scalar per [128,512] pack at PSUM eviction.
  - fc1 PSUM evicts straight through ACT gelu (bias = per-partition AP).
  - softmax normalize: denominator row is PE-broadcast (ones-matmul) and
    inverted with reciprocal_approx_fast (NOT the 3.7us exact reciprocal).
  - phase-level pipelining: V-GEMM fills attn0's exp-wait; fc2 of chunk
    qc-1 fills attn(qc); gelu/exp never interleave (ACT table thrash).
"""

import sys

sys.path.insert(0, "/opt/trn_rl_repo")

import numpy as np
import ml_dtypes

import concourse.bacc as bacc
import concourse.tile as tile
from concourse import mybir
from concourse import bass_utils
from concourse.masks import make_identity

F32 = mybir.dt.float32
BF16 = mybir.dt.bfloat16
FP8 = mybir.dt.float8e4
BF = ml_dtypes.bfloat16
F8NP = ml_dtypes.float8_e4m3
OP = mybir.AluOpType
AF = mybir.ActivationFunctionType

B = 8
L = 1024
D = 1024
H = 16
HD = 64
FF = 4096
LN_EPS = 1e-5
P = 128
TT = L // P
KT = D // P
FT = FF // P
QC = 512
NQC = L // QC

S_QK = 32.0          # q/k fp8 scale
ESC = 0.125 / (S_QK * S_QK)

_CACHE = {}


def _build():
    nc = bacc.Bacc(None, target_bir_lowering=False)
    names = {}
    with tile.TileContext(nc) as tc, \
            tc.tile_pool(name="dram", bufs=1, space="DRAM") as dram, \
            tc.tile_pool(name="per", bufs=1) as per:

        x_d = dram.tile([P, TT, D], F32, kind="ExternalInput", name="x")
        c_d = dram.tile([P, KT], F32, kind="ExternalInput", name="c")
        lnT_d = dram.tile([P, 4 * KT], F32, kind="ExternalInput", name="lnT")
        adaw_d = dram.tile([P, KT, 6 * D], BF16, kind="ExternalInput",
                           name="ada_w")
        adab_d = dram.tile([1, 6 * D], BF16, kind="ExternalInput",
                           name="ada_b")
        qkvw_d = dram.tile([P, KT, 3 * D], BF16, kind="ExternalInput",
                           name="qkv_w")
        bqk_d = dram.tile([P, 16], F32, kind="ExternalInput", name="bqk")
        vbs_d = dram.tile([1, D], BF16, kind="ExternalInput", name="vbs")
        projw_d = dram.tile([P, KT, D], FP8, kind="ExternalInput",
                            name="proj_w")
        pbs_d = dram.tile([1, D], BF16, kind="ExternalInput", name="pbs")
        dqp_d = dram.tile([1, D], BF16, kind="ExternalInput", name="dqp")
        fc1w_d = dram.tile([P, KT, FF], BF16, kind="ExternalInput",
                           name="fc1_w")
        bf1_d = dram.tile([P, FT], F32, kind="ExternalInput", name="bf1")
        fc2w_d = dram.tile([P, FT, D], BF16, kind="ExternalInput",
                           name="fc2_w")
        f2bs_d = dram.tile([1, D], BF16, kind="ExternalInput", name="f2bs")
        out_d = dram.tile([L, D], F32, kind="ExternalOutput", name="out")
        for t, n in [(x_d, "x"), (c_d, "c"), (lnT_d, "lnT"),
                     (adaw_d, "ada_w"), (adab_d, "ada_b"),
                     (qkvw_d, "qkv_w"), (bqk_d, "bqk"), (vbs_d, "vbs"),
                     (projw_d, "proj_w"), (pbs_d, "pbs"), (dqp_d, "dqp"),
                     (fc1w_d, "fc1_w"), (bf1_d, "bf1"), (fc2w_d, "fc2_w"),
                     (f2bs_d, "f2bs"), (out_d, "out")]:
            names[n] = t.name
        out_v = out_d[:].rearrange("(t p) d -> p t d", p=P)

        x_sb = per.tile([P, TT, D], F32)
        for t in range(TT):
            nc.sync.dma_start(x_sb[:, t, :], x_d[:, t, :])
        ident = per.tile([P, P], BF16)
        make_identity(nc, ident[:])
        onesrow = per.tile([1, P], BF16)
        nc.vector.memset(onesrow[:], 1.0)
        eps_sb = per.tile([P, 1], F32)
        nc.vector.memset(eps_sb[:], LN_EPS)

        lnT = per.tile([P, 4 * KT], F32)
        nc.sync.dma_start(lnT[:], lnT_d[:])
        bqk = per.tile([P, 16], F32)
        nc.sync.dma_start(bqk[:], bqk_d[:])
        vbs = per.tile([1, D], BF16)
        nc.sync.dma_start(vbs[:], vbs_d[:])
        pbs = per.tile([1, D], BF16)
        nc.sync.dma_start(pbs[:], pbs_d[:])
        dqp_row = per.tile([1, D], BF16)
        nc.sync.dma_start(dqp_row[:], dqp_d[:])
        bf1 = per.tile([P, FT], F32)
        nc.sync.dma_start(bf1[:], bf1_d[:])
        f2bs = per.tile([1, D], BF16)
        nc.sync.dma_start(f2bs[:], f2bs_d[:])
        adab = per.tile([1, 6 * D], BF16)
        nc.sync.dma_start(adab[:], adab_d[:])
        projw = per.tile([P, KT, D], FP8)
        for k in range(KT):
            nc.sync.dma_start(projw[:, k, :], projw_d[:, k, :])

        eff1s = per.tile([P, KT], F32)
        eff1h = per.tile([P, KT], F32)
        eff2s = per.tile([P, KT], F32)
        eff2h = per.tile([P, KT], F32)
        g1bc = per.tile([P, D], BF16)
        g2bc = per.tile([P, D], BF16)

        q8 = per.tile([P, KT, L], FP8)
        k8 = per.tile([P, KT, L], FP8)
        v_sb = per.tile([P, TT, H, HD + 1], BF16)
        nc.vector.memset(v_sb[:, :, :, HD:HD + 1], 1.0)
        h2T = per.tile([P, KT, QC], BF16)
        geluT = per.tile([P, FT, QC], BF16)

        _work_cm = tc.tile_pool(name="work", bufs=2)
        work = _work_cm.__enter__()

        def ln_stats(x_aps):
            n = len(x_aps)
            mv = work.tile([P, n, 2], F32, tag=f"mv{n}", name=f"mv{n}")
            for i, x_ap in enumerate(x_aps):
                stats = work.tile([P, 2, 6], F32, tag="stats")
                for sg in range(2):
                    nc.vector.bn_stats(stats[:, sg, :],
                                       x_ap[:, sg * 512:(sg + 1) * 512])
                nc.vector.bn_aggr(mv[:, i, :], stats[:])
            rstd = work.tile([P, n], F32, tag=f"rstd{n}", name=f"rstd{n}")
            nc.scalar.activation(rstd[:], mv[:, :, 1], AF.Sqrt, bias=eps_sb[:])
            nc.vector.reciprocal(rstd[:], rstd[:])
            return mv, rstd

        mv1, rstd1 = ln_stats([x_sb[:, t, :] for t in range(TT)])

        # ---------------- ada (bf16, 512-wide moving weights) -----------
        with nc.named_scope("ada"), \
                tc.tile_pool(name="ada_sb", bufs=1) as asb, \
                tc.tile_pool(name="ada_stream", bufs=2) as ast, \
                tc.tile_pool(name="ps_ada", bufs=2, space="PSUM") as psa, \
                tc.tile_pool(name="ps_t6", bufs=1, space="PSUM") as pst6:
            c_sb = asb.tile([P, KT], F32)
            nc.sync.dma_start(c_sb[:], c_d[:])
            silu_b = asb.tile([P, KT], BF16)
            nc.scalar.activation(silu_b[:], c_sb[:], AF.Silu)
            ssgb = asb.tile([1, 6 * D], BF16, name="ssgb")
            for nch in range(12):
                aw = ast.tile([P, KT, 512], BF16, tag="aw")
                for k in range(KT):
                    nc.sync.dma_start(
                        aw[:, k, :],
                        adaw_d[:, k, nch * 512:(nch + 1) * 512])
                pa = psa.tile([1, 512], F32, tag="ada")
                nc.tensor.matmul(pa[:], onesrow[:, 0:1],
                                 adab[:, nch * 512:(nch + 1) * 512],
                                 start=True, stop=False)
                for k in range(KT):
                    nc.tensor.matmul(pa[:], silu_b[:, k:k + 1], aw[:, k, :],
                                     start=False, stop=(k == KT - 1),
                                     skip_group_check=True)
                nc.vector.tensor_copy(
                    ssgb[:, nch * 512:(nch + 1) * 512], pa[:])
            # shift/scale rows -> feature-major cols; v' = sh1,sc1,sh2,sc2
            pt6 = pst6.tile([P, 32, 2], BF16)
            for vi, v in enumerate((0, 1, 3, 4)):
                for ch in range(KT):
                    i = ch * 4 + vi
                    nc.tensor.transpose(
                        pt6[:, i, 0:1],
                        ssgb[0:1, v * D + ch * P:v * D + (ch + 1) * P],
                        ident[0:1, 0:1])
            ssgT = asb.tile([P, KT, 4], F32, name="ssgT")
            nc.vector.tensor_copy(ssgT[:], pt6[:, :, 0])
            t0 = asb.tile([P, KT], F32, name="t0")
            t1 = asb.tile([P, KT], F32, name="t1")
            nc.vector.tensor_scalar_add(t0[:], ssgT[:, :, 1], 1.0)
            nc.vector.tensor_mul(eff1s[:], t0[:], lnT[:, 0:KT])
            nc.vector.tensor_mul(t1[:], t0[:], lnT[:, KT:2 * KT])
            nc.vector.tensor_add(eff1h[:], t1[:], ssgT[:, :, 0])
            nc.vector.tensor_scalar_add(t0[:], ssgT[:, :, 3], 1.0)
            nc.vector.tensor_mul(eff2s[:], t0[:], lnT[:, 2 * KT:3 * KT])
            nc.vector.tensor_mul(t1[:], t0[:], lnT[:, 3 * KT:4 * KT])
            nc.vector.tensor_add(eff2h[:], t1[:], ssgT[:, :, 2])
            g1row = asb.tile([1, D], BF16, name="g1row")
            nc.vector.tensor_mul(g1row[:], ssgb[:, 2 * D:3 * D], dqp_row[:])
            nc.gpsimd.partition_broadcast(g1bc[:], g1row[:])
            nc.gpsimd.partition_broadcast(g2bc[:], ssgb[:, 5 * D:6 * D])

        # ---------------- LN -> hT (transpose + fused modulate) ---------
        def build_hT(dst, eff_s, eff_h, mv, rstd, idx0, tg, toff, pstr):
            norms = []
            for s in range(4):
                t = toff + s
                nrm = work.tile([P, D], BF16, tag="nrm", bufs=4)
                i = idx0 + s
                nc.vector.tensor_scalar(out=nrm[:], in0=x_sb[:, t, :],
                                        scalar1=mv[:, i, 0:1],
                                        scalar2=rstd[:, i:i + 1],
                                        op0=OP.subtract, op1=OP.mult)
                norms.append(nrm)
            for k in range(KT):
                pt = pstr.tile([P, 4 * P], BF16, tag="tr")
                for s in range(4):
                    nc.tensor.transpose(pt[:, s * P:(s + 1) * P],
                                        norms[s][:, k * P:(k + 1) * P],
                                        ident[:])
                nc.vector.tensor_scalar(
                    out=dst[:, k, tg * 512:(tg + 1) * 512], in0=pt[:],
                    scalar1=eff_s[:, k:k + 1], scalar2=eff_h[:, k:k + 1],
                    op0=OP.mult, op1=OP.add)

        # ---------------- prologue: h1T + QKV (bf16, 512-wide) ----------
        pro_d = {}
        wv_half = []

        def open_prologue():
            pro_d["pro_cm"] = tc.tile_pool(name="pro", bufs=1)
            pro = pro_d["pro_cm"].__enter__()
            h1T = pro.tile([P, KT, L], BF16, name="h1T")
            with tc.tile_pool(name="ps_tr1", bufs=2, space="PSUM") as pstr1:
                for tg in range(2):
                    build_hT(h1T, eff1s, eff1h, mv1, rstd1, tg * 4, tg,
                             tg * 4, pstr1)
            pro_d["qs_cm"] = tc.tile_pool(name="qkv_stream", bufs=3)
            pro_d["qst"] = pro_d["qs_cm"].__enter__()
            pro_d["h1T"] = h1T

        def qk_chunk(qk, jj):
            fbase = D if qk == 0 else 0
            dst = k8 if qk == 0 else q8
            wj = pro_d["qst"].tile([P, KT, P], BF16, tag="wj", bufs=2,
                                   name="wjt")
            for k in range(KT):
                nc.sync.dma_start(
                    wj[:, k, :],
                    qkvw_d[:, k, fbase + jj * P:fbase + (jj + 1) * P])
            ci = (fbase + jj * P) // P
            for tg in range(2):
                pq = mm_tile(jj * 2 + tg, "pq")
                for k in range(KT):
                    nc.tensor.matmul(pq[:], wj[:, k, :],
                                     pro_d["h1T"][:, k,
                                                  tg * 512:(tg + 1) * 512],
                                     start=(k == 0), stop=(k == KT - 1))
                nc.vector.tensor_scalar(
                    out=dst[:, jj, tg * 512:(tg + 1) * 512], in0=pq[:],
                    scalar1=S_QK, scalar2=bqk[:, ci:ci + 1],
                    op0=OP.mult, op1=OP.add)

        def v_chunk(tt, fh):
            pv = mm_tile(tt * 2 + fh, "pv")
            nc.tensor.matmul(pv[:], onesrow[:, 0:P],
                             vbs[:, fh * 512:(fh + 1) * 512],
                             start=True, stop=False)
            for k in range(KT):
                nc.tensor.matmul(pv[:],
                                 pro_d["h1T"][:, k, tt * P:(tt + 1) * P],
                                 wv_half[fh][:, k, :],
                                 start=False, stop=(k == KT - 1),
                                 skip_group_check=True)
            nc.vector.tensor_copy(
                v_sb[:, tt, fh * 8:(fh + 1) * 8, 0:HD], pv[:])

        def load_wv(fh):
            wv = pro_d["qst"].tile([P, KT, 512], BF16, tag="wv", bufs=1,
                          name=f"wv{fh}")
            for k in range(KT):
                nc.sync.dma_start(wv[:, k, :],
                                  qkvw_d[:, k, 2 * D + fh * 512:
                                         2 * D + (fh + 1) * 512])
            wv_half.append(wv)

        # ---------------- attention + MLP, phase-pipelined --------------
        fsd = {}

        def fs_tile(*a, **k):
            return fsd["fs"].tile(*a, **k)

        with tc.tile_pool(name="attn", bufs=2) as ap, \
                tc.tile_pool(name="aTp", bufs=1) as aTp, \
                tc.tile_pool(name="ps_s", bufs=2, space="PSUM") as pss, \
                tc.tile_pool(name="ps_av", bufs=1, space="PSUM") as psav, \
                tc.tile_pool(name="ps_m", bufs=1, space="PSUM") as psm:

            def mm_tile(i, nm):
                return psm.tile([P, 512], F32, tag=f"f2_{i % 2}", bufs=1,
                                name=nm)

            open_prologue()
            with nc.named_scope("qkv"):
                for jj in range(KT):
                    qk_chunk(0, jj)      # K
                for jj in range(KT):
                    qk_chunk(1, jj)      # Q
                for fh in range(2):
                    load_wv(fh)
                    for tt in range(TT):
                        v_chunk(tt, fh)

            def scores_exp(qc, j):
                q0 = qc * QC
                attA = ap.tile([P, TT, QC], FP8, tag="attA")
                attB = ap.tile([P, TT, QC], FP8, tag="attB")
                for m in range(TT):
                    psA = pss.tile([P, QC], F32, tag="sc", name="psA")
                    psB = pss.tile([P, QC], F32, tag="sc", name="psB")
                    nc.tensor.matmul(psA[:], k8[0:HD, j, m * P:(m + 1) * P],
                                     q8[0:HD, j, q0:q0 + QC],
                                     start=True, stop=True)
                    nc.tensor.matmul(psB[:], k8[HD:P, j, m * P:(m + 1) * P],
                                     q8[HD:P, j, q0:q0 + QC],
                                     start=True, stop=True,
                                     tile_position=(HD, 0))
                    nc.scalar.activation(attA[:, m, :], psA[:], AF.Exp,
                                         scale=ESC)
                    nc.scalar.activation(attB[:, m, :], psB[:], AF.Exp,
                                         scale=ESC)
                return attA, attB

            def av_norm(j, attA, attB, aT):
                for hh, att in ((0, attA), (1, attB)):
                    h = 2 * j + hh
                    pu = psav.tile([HD + 1, QC], F32, tag="pu")
                    for m in range(TT):
                        nc.tensor.matmul(pu[:], v_sb[:, m, h, :],
                                         att[:, m, :],
                                         start=(m == 0), stop=(m == TT - 1))
                    drow = work.tile([1, QC], BF16, tag="drow", bufs=1)
                    nc.vector.tensor_copy(drow[:], pu[HD:HD + 1, :])
                    rb = psav.tile([HD, QC], F32, tag="rb")
                    nc.tensor.matmul(rb[:], onesrow[:, 0:HD], drow[:],
                                     start=True, stop=True)
                    rec = work.tile([HD, QC], F32, tag="rec", bufs=1)
                    nc.vector.reciprocal_approx_fast(rec[:], rb[:])
                    nc.vector.tensor_tensor(
                        out=aT[hh * 64:(hh + 1) * 64, j, :],
                        in0=pu[0:HD, :], in1=rec[:], op=OP.mult)

            def proj_block(qc, aT, s):
                t_global = qc * 4 + s
                for fh in range(2):
                    pp = mm_tile(s * 2 + fh, "pp")
                    nc.tensor.matmul(pp[:], onesrow[:, 0:P],
                                     pbs[:, fh * 512:(fh + 1) * 512],
                                     start=True, stop=False)
                    for k in range(KT):
                        nc.tensor.matmul(
                            pp[:], aT[:, k, s * P:(s + 1) * P],
                            projw[:, k, fh * 512:(fh + 1) * 512],
                            start=False, stop=(k == KT - 1),
                            skip_group_check=True)
                    tmp = work.tile([P, 512], BF16, tag="ptmp", bufs=1)
                    nc.vector.tensor_tensor(
                        out=tmp[:], in0=pp[:],
                        in1=g1bc[:, fh * 512:(fh + 1) * 512], op=OP.mult)
                    nc.vector.tensor_add(
                        x_sb[:, t_global, fh * 512:(fh + 1) * 512],
                        x_sb[:, t_global, fh * 512:(fh + 1) * 512],
                        tmp[:])

            def fc1_gelu(qc):
                for fc in range(FT):
                    w1 = fs_tile([P, KT, P], BF16, tag="w1", bufs=4,
                                 name="w1t")
                    for k in range(KT):
                        nc.sync.dma_start(
                            w1[:, k, :], fc1w_d[:, k, fc * P:(fc + 1) * P])
                    pf = mm_tile(fc, "pf")
                    for k in range(KT):
                        nc.tensor.matmul(pf[:], w1[:, k, :], h2T[:, k, :],
                                         start=(k == 0), stop=(k == KT - 1))
                    nc.scalar.activation(geluT[:, fc, :], pf[:],
                                         AF.Gelu_apprx_tanh,
                                         bias=bf1[:, fc:fc + 1])

            def fc2_blocks(qc):
                # s-pairs share the streamed w2 chunk; 2 psum banks live
                for sp in range(2):
                    for fp in range(2):
                        def blk(sp=sp, fp=fp, qc=qc):
                            ps2 = [mm_tile(0, "pf2a"), mm_tile(1, "pf2b")]
                            for s in range(2):
                                nc.tensor.matmul(
                                    ps2[s][:], onesrow[:, 0:P],
                                    f2bs[:, fp * 512:(fp + 1) * 512],
                                    start=True, stop=False)
                            for ft in range(FT):
                                w2 = fs_tile([P, 512], BF16, tag="w2",
                                             bufs=12, name="w2t")
                                nc.sync.dma_start(
                                    w2[:], fc2w_d[:, ft, fp * 512:
                                                  (fp + 1) * 512])
                                for s in range(2):
                                    tok = sp * 2 + s
                                    nc.tensor.matmul(
                                        ps2[s][:],
                                        geluT[:, ft,
                                              tok * P:(tok + 1) * P],
                                        w2[:],
                                        start=False, stop=(ft == FT - 1),
                                        skip_group_check=True)
                            for s in range(2):
                                t_global = qc * 4 + sp * 2 + s
                                tmp = work.tile([P, 512], BF16, tag="ftmp",
                                                bufs=1)
                                nc.vector.tensor_tensor(
                                    out=tmp[:], in0=ps2[s][:],
                                    in1=g2bc[:, fp * 512:(fp + 1) * 512],
                                    op=OP.mult)
                                oth = work.tile([P, 512], F32, tag="ot",
                                                bufs=2,
                                                name=f"ot{qc}{fp}{sp}{s}")
                                nc.vector.tensor_add(
                                    oth[:],
                                    x_sb[:, t_global,
                                         fp * 512:(fp + 1) * 512],
                                    tmp[:])
                                nc.sync.dma_start(
                                    out_v[:, t_global,
                                          fp * 512:(fp + 1) * 512],
                                    oth[:])
                        yield blk

            closed = {}

            def run_qc(qc, pending):
                pend_i = 0
                with nc.named_scope(f"attn{qc}"):
                    aT = aTp.tile([P, KT, QC], BF16, tag="aT",
                                  name=f"aT_{qc}")
                    atts = []
                    for j in range(KT):
                        atts.append(scores_exp(qc, j))
                        for _ in range(2):
                            if pend_i < len(pending):
                                pending[pend_i]()
                                pend_i += 1
                        if j >= 1:
                            av_norm(j - 1, *atts[j - 1], aT)
                    av_norm(KT - 1, *atts[KT - 1], aT)
                if qc == 0 and not closed:
                    # h1T / V weights are dead: free prologue SBUF
                    pro_d["qs_cm"].__exit__(None, None, None)
                    pro_d["pro_cm"].__exit__(None, None, None)
                    fsd["cm"] = tc.tile_pool(name="fc_stream", bufs=2)
                    fsd["fs"] = fsd["cm"].__enter__()
                    closed["done"] = True
                with nc.named_scope(f"proj{qc}"):
                    for s in range(4):
                        proj_block(qc, aT, s)
                with nc.named_scope(f"ln2_{qc}"):
                    mv2, rstd2 = ln_stats(
                        [x_sb[:, qc * 4 + s, :] for s in range(4)])
                    with tc.tile_pool(name="ps_tr2", bufs=2,
                                      space="PSUM") as pstr2:
                        build_hT(h2T, eff2s, eff2h, mv2, rstd2, 0, 0,
                                 qc * 4, pstr2)
                with nc.named_scope(f"fc1_{qc}"):
                    fc1_gelu(qc)
                    while pend_i < len(pending):
                        pending[pend_i]()
                        pend_i += 1
                return list(fc2_blocks(qc))

            pending = run_qc(0, [])
            pending = run_qc(1, pending)
            with nc.named_scope("mlp_tail"):
                for blk in pending:
                    blk()
            fsd["cm"].__exit__(None, None, None)

        _work_cm.__exit__(None, None, None)

    nc.compile()
    return nc, names


def _get_compiled():
    if "nc" not in _CACHE:
        _CACHE["nc"], _CACHE["names"] = _build()
    return _CACHE["nc"], _CACHE["names"]


def _q8col(w):
    w = np.asarray(w, np.float32)
    am = np.abs(w).max(axis=0, keepdims=True)
    s = np.where(am > 0, 224.0 / np.maximum(am, 1e-30), 1.0)
    w8 = np.clip(w * s, -240, 240).astype(F8NP)
    return w8, s[0]


def _pmajor(w):
    w = np.asarray(w)
    kp, n = w.shape
    return np.ascontiguousarray(w.reshape(kp // P, P, n).transpose(1, 0, 2))


def _prep_maps(names, x, c, ln1_w, ln1_b, ln2_w, ln2_b, ada_w, ada_b,
               qkv_w, qkv_b, proj_w, proj_b, fc1_w, fc1_b, fc2_w, fc2_b):
    x = np.asarray(x, np.float32)
    c = np.asarray(c, np.float32)
    proj8, s_proj = _q8col(proj_w)
    qkv_b = np.asarray(qkv_b, np.float32)

    def tcols(v):
        return np.asarray(v, np.float32).reshape(KT, P).T
    lnT = np.concatenate([tcols(ln1_w), tcols(ln1_b),
                          tcols(ln2_w), tcols(ln2_b)], axis=1)

    common = {
        names["lnT"]: np.ascontiguousarray(lnT, np.float32),
        names["ada_w"]: _pmajor(ada_w).astype(BF),
        names["ada_b"]: np.asarray(ada_b).astype(BF).reshape(1, -1),
        names["qkv_w"]: _pmajor(qkv_w).astype(BF),
        names["bqk"]: np.ascontiguousarray(
            (qkv_b[0:2 * D] * S_QK).reshape(16, P).T.astype(np.float32)),
        names["vbs"]: qkv_b[2 * D:].astype(BF).reshape(1, D),
        names["proj_w"]: _pmajor(proj8),
        names["pbs"]: (np.asarray(proj_b, np.float32) * s_proj)
        .astype(BF).reshape(1, D),
        names["dqp"]: (1.0 / s_proj).astype(BF).reshape(1, D),
        names["fc1_w"]: _pmajor(fc1_w).astype(BF),
        names["bf1"]: np.ascontiguousarray(
            np.asarray(fc1_b, np.float32).reshape(FT, P).T),
        names["fc2_w"]: _pmajor(fc2_w).astype(BF),
        names["f2bs"]: np.asarray(fc2_b).astype(BF).reshape(1, D),
    }
    in_maps = []
    for b in range(B):
        m = dict(common)
        m[names["x"]] = np.ascontiguousarray(
            x[b].reshape(TT, P, D).transpose(1, 0, 2))
        m[names["c"]] = np.ascontiguousarray(c[b].reshape(KT, P).T)
        in_maps.append(m)
    return in_maps


def kernel(x, c, ln1_w, ln1_b, ln2_w, ln2_b, ada_w, ada_b,
           qkv_w, qkv_b, proj_w, proj_b, fc1_w, fc1_b, fc2_w, fc2_b,
           _trace=False):
    nc, names = _get_compiled()
    in_maps = _prep_maps(names, x, c, ln1_w, ln1_b, ln2_w, ln2_b,
                         ada_w, ada_b, qkv_w, qkv_b, proj_w, proj_b,
                         fc1_w, fc1_b, fc2_w, fc2_b)
    res = bass_utils.run_bass_kernel_spmd(nc, in_maps, core_ids=list(range(B)),
                                          trace=_trace)
    out = np.stack([res.results[b][names["out"]] for b in range(B)])
    if _trace:
        _CACHE["last_result"] = res
    return out


# revision 33
# speedup vs baseline: 1.0348x; 1.0348x over previous
"""DiT block kernel for 8 Trainium2 NeuronCores.

Sharding: data-parallel over batch (B=8 -> one batch element per core).

Vs the original bf16 baseline:
  - scores / ada / fc1 / fc2 use 512-wide moving operands (baseline used
    256) — wide streams hide the PE weight-load latency.
  - q/k/attn stored fp8(e4m3): halves SBUF + eviction cost; scores and
    attn@V run with fp8 moving operands at bf16 rate; proj streams fp8
    per-column-quantized weights (dequant folded into the gate tile).
  - LN: token-major norm -> PE transpose -> modulate fused into ONE
    tensor_scalar per [128,512] pack at PSUM eviction.
  - fc1 PSUM evicts straight through ACT gelu (bias = per-partition AP).
  - softmax normalize: denominator row is PE-broadcast (ones-matmul) and
    inverted with reciprocal_approx_fast (NOT the 3.7us exact reciprocal).
  - phase-level pipelining: V-GEMM fills attn0's exp-wait; fc2 of chunk
    qc-1 fills attn(qc); gelu/exp never interleave (ACT table thrash).
"""

import sys

sys.path.insert(0, "/opt/trn_rl_repo")

import numpy as np
import ml_dtypes

import concourse.bacc as bacc
import concourse.tile as tile
from concourse import mybir
from concourse import bass_utils
from concourse.masks import make_identity

F32 = mybir.dt.float32
BF16 = mybir.dt.bfloat16
FP8 = mybir.dt.float8e4
BF = ml_dtypes.bfloat16
F8NP = ml_dtypes.float8_e4m3
OP = mybir.AluOpType
AF = mybir.ActivationFunctionType

B = 8
L = 1024
D = 1024
H = 16
HD = 64
FF = 4096
LN_EPS = 1e-5
P = 128
TT = L // P
KT = D // P
FT = FF // P
QC = 512
NQC = L // QC

S_QK = 32.0          # q/k fp8 scale
ESC = 0.125 / (S_QK * S_QK)

_CACHE = {}


def _build():
    nc = bacc.Bacc(None, target_bir_lowering=False)
    names = {}
    with tile.TileContext(nc) as tc, \
            tc.tile_pool(name="dram", bufs=1, space="DRAM") as dram, \
            tc.tile_pool(name="per", bufs=1) as per:

        x_d = dram.tile([P, TT, D], F32, kind="ExternalInput", name="x")
        c_d = dram.tile([P, KT], F32, kind="ExternalInput", name="c")
        lnT_d = dram.tile([P, 4 * KT], F32, kind="ExternalInput", name="lnT")
        adaw_d = dram.tile([P, KT, 6 * D], BF16, kind="ExternalInput",
                           name="ada_w")
        adab_d = dram.tile([1, 6 * D], BF16, kind="ExternalInput",
                           name="ada_b")
        qkvw_d = dram.tile([P, KT, 3 * D], BF16, kind="ExternalInput",
                           name="qkv_w")
        bqk_d = dram.tile([P, 16], F32, kind="ExternalInput", name="bqk")
        vbs_d = dram.tile([1, D], BF16, kind="ExternalInput", name="vbs")
        projw_d = dram.tile([P, KT, D], FP8, kind="ExternalInput",
                            name="proj_w")
        pbs_d = dram.tile([1, D], BF16, kind="ExternalInput", name="pbs")
        dqp_d = dram.tile([1, D], BF16, kind="ExternalInput", name="dqp")
        fc1w_d = dram.tile([P, KT, FF], BF16, kind="ExternalInput",
                           name="fc1_w")
        bf1_d = dram.tile([P, FT], F32, kind="ExternalInput", name="bf1")
        fc2w_d = dram.tile([P, FT, D], BF16, kind="ExternalInput",
                           name="fc2_w")
        f2bs_d = dram.tile([1, D], BF16, kind="ExternalInput", name="f2bs")
        out_d = dram.tile([L, D], F32, kind="ExternalOutput", name="out")
        for t, n in [(x_d, "x"), (c_d, "c"), (lnT_d, "lnT"),
                     (adaw_d, "ada_w"), (adab_d, "ada_b"),
                     (qkvw_d, "qkv_w"), (bqk_d, "bqk"), (vbs_d, "vbs"),
                     (projw_d, "proj_w"), (pbs_d, "pbs"), (dqp_d, "dqp"),
                     (fc1w_d, "fc1_w"), (bf1_d, "bf1"), (fc2w_d, "fc2_w"),
                     (f2bs_d, "f2bs"), (out_d, "out")]:
            names[n] = t.name
        out_v = out_d[:].rearrange("(t p) d -> p t d", p=P)

        x_sb = per.tile([P, TT, D], F32)
        for t in range(TT):
            nc.sync.dma_start(x_sb[:, t, :], x_d[:, t, :])
        ident = per.tile([P, P], BF16)
        make_identity(nc, ident[:])
        onesrow = per.tile([1, P], BF16)
        nc.vector.memset(onesrow[:], 1.0)
        eps_sb = per.tile([P, 1], F32)
        nc.vector.memset(eps_sb[:], LN_EPS)

        lnT = per.tile([P, 4 * KT], F32)
        nc.sync.dma_start(lnT[:], lnT_d[:])
        bqk = per.tile([P, 16], F32)
        nc.sync.dma_start(bqk[:], bqk_d[:])
        vbs = per.tile([1, D], BF16)
        nc.sync.dma_start(vbs[:], vbs_d[:])
        pbs = per.tile([1, D], BF16)
        nc.sync.dma_start(pbs[:], pbs_d[:])
        dqp_row = per.tile([1, D], BF16)
        nc.sync.dma_start(dqp_row[:], dqp_d[:])
        bf1 = per.tile([P, FT], F32)
        nc.sync.dma_start(bf1[:], bf1_d[:])
        f2bs = per.tile([1, D], BF16)
        nc.sync.dma_start(f2bs[:], f2bs_d[:])
        adab = per.tile([1, 6 * D], BF16)
        nc.sync.dma_start(adab[:], adab_d[:])
        projw = per.tile([P, KT, D], FP8)
        for k in range(KT):
            nc.sync.dma_start(projw[:, k, :], projw_d[:, k, :])

        eff1s = per.tile([P, KT], F32)
        eff1h = per.tile([P, KT], F32)
        eff2s = per.tile([P, KT], F32)
        eff2h = per.tile([P, KT], F32)
        g1bc = per.tile([P, D], BF16)
        g2bc = per.tile([P, D], BF16)

        q8 = per.tile([P, KT, L], FP8)
        k8 = per.tile([P, KT, L], FP8)
        v_sb = per.tile([P, TT, H, HD + 1], BF16)
        nc.vector.memset(v_sb[:, :, :, HD:HD + 1], 1.0)
        h2T = per.tile([P, KT, QC], BF16)
        geluT = per.tile([P, FT, QC], BF16)

        _work_cm = tc.tile_pool(name="work", bufs=2)
        work = _work_cm.__enter__()

        def ln_stats(x_aps):
            n = len(x_aps)
            mv = work.tile([P, n, 2], F32, tag=f"mv{n}", name=f"mv{n}")
            for i, x_ap in enumerate(x_aps):
                stats = work.tile([P, 2, 6], F32, tag="stats")
                for sg in range(2):
                    nc.vector.bn_stats(stats[:, sg, :],
                                       x_ap[:, sg * 512:(sg + 1) * 512])
                nc.vector.bn_aggr(mv[:, i, :], stats[:])
            rstd = work.tile([P, n], F32, tag=f"rstd{n}", name=f"rstd{n}")
            nc.scalar.activation(rstd[:], mv[:, :, 1], AF.Sqrt, bias=eps_sb[:])
            nc.vector.reciprocal(rstd[:], rstd[:])
            return mv, rstd

        mv1, rstd1 = ln_stats([x_sb[:, t, :] for t in range(TT)])

        # ---------------- ada (bf16, 512-wide moving weights) -----------
        with nc.named_scope("ada"), \
                tc.tile_pool(name="ada_sb", bufs=1) as asb, \
                tc.tile_pool(name="ada_stream", bufs=3) as ast, \
                tc.tile_pool(name="ps_ada", bufs=2, space="PSUM") as psa, \
                tc.tile_pool(name="ps_t6", bufs=1, space="PSUM") as pst6:
            c_sb = asb.tile([P, KT], F32)
            nc.sync.dma_start(c_sb[:], c_d[:])
            silu_b = asb.tile([P, KT], BF16)
            nc.scalar.activation(silu_b[:], c_sb[:], AF.Silu)
            ssgb = asb.tile([1, 6 * D], BF16, name="ssgb")
            for nch in range(12):
                aw = ast.tile([P, KT, 512], BF16, tag="aw")
                nc.sync.dma_start(aw[:],
                                  adaw_d[:, :, nch * 512:(nch + 1) * 512])
                pa = psa.tile([1, 512], F32, tag="ada")
                nc.tensor.matmul(pa[:], onesrow[:, 0:1],
                                 adab[:, nch * 512:(nch + 1) * 512],
                                 start=True, stop=False)
                for k in range(KT):
                    nc.tensor.matmul(pa[:], silu_b[:, k:k + 1], aw[:, k, :],
                                     start=False, stop=(k == KT - 1),
                                     skip_group_check=True)
                nc.vector.tensor_copy(
                    ssgb[:, nch * 512:(nch + 1) * 512], pa[:])
            # shift/scale rows -> feature-major cols; v' = sh1,sc1,sh2,sc2
            pt6 = pst6.tile([P, 32, 2], BF16)
            for vi, v in enumerate((0, 1, 3, 4)):
                for ch in range(KT):
                    i = ch * 4 + vi
                    nc.tensor.transpose(
                        pt6[:, i, 0:1],
                        ssgb[0:1, v * D + ch * P:v * D + (ch + 1) * P],
                        ident[0:1, 0:1])
            ssgT = asb.tile([P, KT, 4], F32, name="ssgT")
            nc.vector.tensor_copy(ssgT[:], pt6[:, :, 0])
            t0 = asb.tile([P, KT], F32, name="t0")
            t1 = asb.tile([P, KT], F32, name="t1")
            nc.vector.tensor_scalar_add(t0[:], ssgT[:, :, 1], 1.0)
            nc.vector.tensor_mul(eff1s[:], t0[:], lnT[:, 0:KT])
            nc.vector.tensor_mul(t1[:], t0[:], lnT[:, KT:2 * KT])
            nc.vector.tensor_add(eff1h[:], t1[:], ssgT[:, :, 0])
            nc.vector.tensor_scalar_add(t0[:], ssgT[:, :, 3], 1.0)
            nc.vector.tensor_mul(eff2s[:], t0[:], lnT[:, 2 * KT:3 * KT])
            nc.vector.tensor_mul(t1[:], t0[:], lnT[:, 3 * KT:4 * KT])
            nc.vector.tensor_add(eff2h[:], t1[:], ssgT[:, :, 2])
            g1row = asb.tile([1, D], BF16, name="g1row")
            nc.vector.tensor_mul(g1row[:], ssgb[:, 2 * D:3 * D], dqp_row[:])
            nc.gpsimd.partition_broadcast(g1bc[:], g1row[:])
            nc.gpsimd.partition_broadcast(g2bc[:], ssgb[:, 5 * D:6 * D])

        # ---------------- LN -> hT (transpose + fused modulate) ---------
        def build_hT(dst, eff_s, eff_h, mv, rstd, idx0, tg, toff, pstr):
            norms = []
            for s in range(4):
                t = toff + s
                nrm = work.tile([P, D], BF16, tag="nrm", bufs=4)
                i = idx0 + s
                nc.vector.tensor_scalar(out=nrm[:], in0=x_sb[:, t, :],
                                        scalar1=mv[:, i, 0:1],
                                        scalar2=rstd[:, i:i + 1],
                                        op0=OP.subtract, op1=OP.mult)
                norms.append(nrm)
            for k in range(KT):
                pt = pstr.tile([P, 4 * P], BF16, tag="tr")
                for s in range(4):
                    nc.tensor.transpose(pt[:, s * P:(s + 1) * P],
                                        norms[s][:, k * P:(k + 1) * P],
                                        ident[:])
                nc.vector.tensor_scalar(
                    out=dst[:, k, tg * 512:(tg + 1) * 512], in0=pt[:],
                    scalar1=eff_s[:, k:k + 1], scalar2=eff_h[:, k:k + 1],
                    op0=OP.mult, op1=OP.add)

        # ---------------- prologue: h1T + QKV (bf16, 512-wide) ----------
        pro_d = {}
        wv_half = []

        def open_prologue():
            pro_d["pro_cm"] = tc.tile_pool(name="pro", bufs=1)
            pro = pro_d["pro_cm"].__enter__()
            h1T = pro.tile([P, KT, L], BF16, name="h1T")
            with tc.tile_pool(name="ps_tr1", bufs=2, space="PSUM") as pstr1:
                for tg in range(2):
                    build_hT(h1T, eff1s, eff1h, mv1, rstd1, tg * 4, tg,
                             tg * 4, pstr1)
            pro_d["qs_cm"] = tc.tile_pool(name="qkv_stream", bufs=3)
            pro_d["qst"] = pro_d["qs_cm"].__enter__()
            pro_d["h1T"] = h1T

        def qk_chunk(qk, jj):
            fbase = D if qk == 0 else 0
            dst = k8 if qk == 0 else q8
            wj = pro_d["qst"].tile([P, KT, P], BF16, tag="wj", bufs=2,
                                   name="wjt")
            nc.sync.dma_start(
                wj[:], qkvw_d[:, :, fbase + jj * P:fbase + (jj + 1) * P])
            ci = (fbase + jj * P) // P
            for tg in range(2):
                pq = mm_tile(jj * 2 + tg, "pq")
                for k in range(KT):
                    nc.tensor.matmul(pq[:], wj[:, k, :],
                                     pro_d["h1T"][:, k,
                                                  tg * 512:(tg + 1) * 512],
                                     start=(k == 0), stop=(k == KT - 1))
                nc.vector.tensor_scalar(
                    out=dst[:, jj, tg * 512:(tg + 1) * 512], in0=pq[:],
                    scalar1=S_QK, scalar2=bqk[:, ci:ci + 1],
                    op0=OP.mult, op1=OP.add)

        def v_chunk(tt, fh):
            pv = mm_tile(tt * 2 + fh, "pv")
            nc.tensor.matmul(pv[:], onesrow[:, 0:P],
                             vbs[:, fh * 512:(fh + 1) * 512],
                             start=True, stop=False)
            for k in range(KT):
                nc.tensor.matmul(pv[:],
                                 pro_d["h1T"][:, k, tt * P:(tt + 1) * P],
                                 wv_half[fh][:, k, :],
                                 start=False, stop=(k == KT - 1),
                                 skip_group_check=True)
            nc.vector.tensor_copy(
                v_sb[:, tt, fh * 8:(fh + 1) * 8, 0:HD], pv[:])

        def load_wv(fh):
            wv = pro_d["qst"].tile([P, KT, 512], BF16, tag="wv", bufs=1,
                          name=f"wv{fh}")
            nc.sync.dma_start(wv[:],
                              qkvw_d[:, :, 2 * D + fh * 512:
                                     2 * D + (fh + 1) * 512])
            wv_half.append(wv)

        # ---------------- attention + MLP, phase-pipelined --------------
        fsd = {}

        def fs_tile(*a, **k):
            return fsd["fs"].tile(*a, **k)

        with tc.tile_pool(name="attn", bufs=2) as ap, \
                tc.tile_pool(name="aTp", bufs=1) as aTp, \
                tc.tile_pool(name="ps_s", bufs=2, space="PSUM") as pss, \
                tc.tile_pool(name="ps_av", bufs=1, space="PSUM") as psav, \
                tc.tile_pool(name="ps_m", bufs=1, space="PSUM") as psm:

            def mm_tile(i, nm):
                return psm.tile([P, 512], F32, tag=f"f2_{i % 2}", bufs=1,
                                name=nm)

            open_prologue()
            with nc.named_scope("qkv"):
                for jj in range(KT):
                    qk_chunk(0, jj)      # K
                for jj in range(KT):
                    qk_chunk(1, jj)      # Q
                load_wv(0)

            def scores_exp(qc, j):
                q0 = qc * QC
                attA = ap.tile([P, TT, QC], FP8, tag="attA")
                attB = ap.tile([P, TT, QC], FP8, tag="attB")
                for m in range(TT):
                    psA = pss.tile([P, QC], F32, tag="sc", name="psA")
                    psB = pss.tile([P, QC], F32, tag="sc", name="psB")
                    nc.tensor.matmul(psA[:], k8[0:HD, j, m * P:(m + 1) * P],
                                     q8[0:HD, j, q0:q0 + QC],
                                     start=True, stop=True)
                    nc.tensor.matmul(psB[:], k8[HD:P, j, m * P:(m + 1) * P],
                                     q8[HD:P, j, q0:q0 + QC],
                                     start=True, stop=True,
                                     tile_position=(HD, 0))
                    nc.scalar.activation(attA[:, m, :], psA[:], AF.Exp,
                                         scale=ESC)
                    nc.scalar.activation(attB[:, m, :], psB[:], AF.Exp,
                                         scale=ESC)
                return attA, attB

            def av_norm(j, attA, attB, aT):
                for hh, att in ((0, attA), (1, attB)):
                    h = 2 * j + hh
                    pu = psav.tile([HD + 1, QC], F32, tag="pu")
                    for m in range(TT):
                        nc.tensor.matmul(pu[:], v_sb[:, m, h, :],
                                         att[:, m, :],
                                         start=(m == 0), stop=(m == TT - 1))
                    drow = work.tile([1, QC], BF16, tag="drow", bufs=1)
                    nc.vector.tensor_copy(drow[:], pu[HD:HD + 1, :])
                    rb = psav.tile([HD, QC], F32, tag="rb")
                    nc.tensor.matmul(rb[:], onesrow[:, 0:HD], drow[:],
                                     start=True, stop=True)
                    rec = work.tile([HD, QC], F32, tag="rec", bufs=1)
                    nc.vector.reciprocal_approx_fast(rec[:], rb[:])
                    nc.vector.tensor_tensor(
                        out=aT[hh * 64:(hh + 1) * 64, j, :],
                        in0=pu[0:HD, :], in1=rec[:], op=OP.mult)

            def proj_block(qc, aT, s):
                t_global = qc * 4 + s
                for fh in range(2):
                    pp = mm_tile(s * 2 + fh, "pp")
                    nc.tensor.matmul(pp[:], onesrow[:, 0:P],
                                     pbs[:, fh * 512:(fh + 1) * 512],
                                     start=True, stop=False)
                    for k in range(KT):
                        nc.tensor.matmul(
                            pp[:], aT[:, k, s * P:(s + 1) * P],
                            projw[:, k, fh * 512:(fh + 1) * 512],
                            start=False, stop=(k == KT - 1),
                            skip_group_check=True)
                    tmp = work.tile([P, 512], BF16, tag="ptmp", bufs=1)
                    nc.vector.tensor_tensor(
                        out=tmp[:], in0=pp[:],
                        in1=g1bc[:, fh * 512:(fh + 1) * 512], op=OP.mult)
                    nc.vector.tensor_add(
                        x_sb[:, t_global, fh * 512:(fh + 1) * 512],
                        x_sb[:, t_global, fh * 512:(fh + 1) * 512],
                        tmp[:])

            def fc1_gelu(qc):
                for fc in range(FT):
                    w1 = fs_tile([P, KT, P], BF16, tag="w1", bufs=3,
                                 name="w1t")
                    nc.sync.dma_start(w1[:],
                                      fc1w_d[:, :, fc * P:(fc + 1) * P])
                    pf = mm_tile(fc, "pf")
                    for k in range(KT):
                        nc.tensor.matmul(pf[:], w1[:, k, :], h2T[:, k, :],
                                         start=(k == 0), stop=(k == KT - 1))
                    nc.scalar.activation(geluT[:, fc, :], pf[:],
                                         AF.Gelu_apprx_tanh,
                                         bias=bf1[:, fc:fc + 1])

            def fc2_blocks(qc):
                # s-pairs share the streamed w2 chunk; 2 psum banks live
                for sp in range(2):
                    for fp in range(2):
                        def blk(sp=sp, fp=fp, qc=qc):
                            ps2 = [mm_tile(0, "pf2a"), mm_tile(1, "pf2b")]
                            for s in range(2):
                                nc.tensor.matmul(
                                    ps2[s][:], onesrow[:, 0:P],
                                    f2bs[:, fp * 512:(fp + 1) * 512],
                                    start=True, stop=False)
                            for ft in range(FT):
                                w2 = fs_tile([P, 512], BF16, tag="w2",
                                             bufs=3, name="w2t")
                                nc.sync.dma_start(
                                    w2[:], fc2w_d[:, ft, fp * 512:
                                                  (fp + 1) * 512])
                                for s in range(2):
                                    tok = sp * 2 + s
                                    nc.tensor.matmul(
                                        ps2[s][:],
                                        geluT[:, ft,
                                              tok * P:(tok + 1) * P],
                                        w2[:],
                                        start=False, stop=(ft == FT - 1),
                                        skip_group_check=True)
                            for s in range(2):
                                t_global = qc * 4 + sp * 2 + s
                                tmp = work.tile([P, 512], BF16, tag="ftmp",
                                                bufs=1)
                                nc.vector.tensor_tensor(
                                    out=tmp[:], in0=ps2[s][:],
                                    in1=g2bc[:, fp * 512:(fp + 1) * 512],
                                    op=OP.mult)
                                oth = work.tile([P, 512], F32, tag="ot",
                                                bufs=2,
                                                name=f"ot{qc}{fp}{sp}{s}")
                                nc.vector.tensor_add(
                                    oth[:],
                                    x_sb[:, t_global,
                                         fp * 512:(fp + 1) * 512],
                                    tmp[:])
                                nc.sync.dma_start(
                                    out_v[:, t_global,
                                          fp * 512:(fp + 1) * 512],
                                    oth[:])
                        yield blk

            closed = {}

            def run_qc(qc, pending):
                pend_i = 0
                with nc.named_scope(f"attn{qc}"):
                    aT = aTp.tile([P, KT, QC], BF16, tag="aT",
                                  name=f"aT_{qc}")
                    atts = []
                    for j in range(KT):
                        atts.append(scores_exp(qc, j))
                        if qc == 0 and j == 0:
                            # V GEMM fills the exp-wait; fully issued
                            # BEFORE any av_norm so deps are complete
                            for fh in range(2):
                                if fh == 1:
                                    load_wv(1)
                                for tt in range(TT):
                                    v_chunk(tt, fh)
                        for _ in range(2):
                            if pend_i < len(pending):
                                pending[pend_i]()
                                pend_i += 1
                        if j >= 1:
                            av_norm(j - 1, *atts[j - 1], aT)
                    av_norm(KT - 1, *atts[KT - 1], aT)
                if qc == 0 and not closed:
                    # h1T / V weights are dead: free prologue SBUF
                    pro_d["qs_cm"].__exit__(None, None, None)
                    pro_d["pro_cm"].__exit__(None, None, None)
                    fsd["cm"] = tc.tile_pool(name="fc_stream", bufs=2)
                    fsd["fs"] = fsd["cm"].__enter__()
                    closed["done"] = True
                with nc.named_scope(f"proj{qc}"):
                    for s in range(4):
                        proj_block(qc, aT, s)
                with nc.named_scope(f"ln2_{qc}"):
                    mv2, rstd2 = ln_stats(
                        [x_sb[:, qc * 4 + s, :] for s in range(4)])
                    with tc.tile_pool(name="ps_tr2", bufs=2,
                                      space="PSUM") as pstr2:
                        build_hT(h2T, eff2s, eff2h, mv2, rstd2, 0, 0,
                                 qc * 4, pstr2)
                with nc.named_scope(f"fc1_{qc}"):
                    fc1_gelu(qc)
                    while pend_i < len(pending):
                        pending[pend_i]()
                        pend_i += 1
                return list(fc2_blocks(qc))

            pending = run_qc(0, [])
            pending = run_qc(1, pending)
            with nc.named_scope("mlp_tail"):
                for blk in pending:
                    blk()
            fsd["cm"].__exit__(None, None, None)

        _work_cm.__exit__(None, None, None)

    nc.compile()
    return nc, names


def _get_compiled():
    if "nc" not in _CACHE:
        _CACHE["nc"], _CACHE["names"] = _build()
    return _CACHE["nc"], _CACHE["names"]


def _q8col(w):
    w = np.asarray(w, np.float32)
    am = np.abs(w).max(axis=0, keepdims=True)
    s = np.where(am > 0, 224.0 / np.maximum(am, 1e-30), 1.0)
    w8 = np.clip(w * s, -240, 240).astype(F8NP)
    return w8, s[0]


def _pmajor(w):
    w = np.asarray(w)
    kp, n = w.shape
    return np.ascontiguousarray(w.reshape(kp // P, P, n).transpose(1, 0, 2))


def _prep_maps(names, x, c, ln1_w, ln1_b, ln2_w, ln2_b, ada_w, ada_b,
               qkv_w, qkv_b, proj_w, proj_b, fc1_w, fc1_b, fc2_w, fc2_b):
    x = np.asarray(x, np.float32)
    c = np.asarray(c, np.float32)
    proj8, s_proj = _q8col(proj_w)
    qkv_b = np.asarray(qkv_b, np.float32)

    def tcols(v):
        return np.asarray(v, np.float32).reshape(KT, P).T
    lnT = np.concatenate([tcols(ln1_w), tcols(ln1_b),
                          tcols(ln2_w), tcols(ln2_b)], axis=1)

    common = {
        names["lnT"]: np.ascontiguousarray(lnT, np.float32),
        names["ada_w"]: _pmajor(ada_w).astype(BF),
        names["ada_b"]: np.asarray(ada_b).astype(BF).reshape(1, -1),
        names["qkv_w"]: _pmajor(qkv_w).astype(BF),
        names["bqk"]: np.ascontiguousarray(
            (qkv_b[0:2 * D] * S_QK).reshape(16, P).T.astype(np.float32)),
        names["vbs"]: qkv_b[2 * D:].astype(BF).reshape(1, D),
        names["proj_w"]: _pmajor(proj8),
        names["pbs"]: (np.asarray(proj_b, np.float32) * s_proj)
        .astype(BF).reshape(1, D),
        names["dqp"]: (1.0 / s_proj).astype(BF).reshape(1, D),
        names["fc1_w"]: _pmajor(fc1_w).astype(BF),
        names["bf1"]: np.ascontiguousarray(
            np.asarray(fc1_b, np.float32).reshape(FT, P).T),
        names["fc2_w"]: _pmajor(fc2_w).astype(BF),
        names["f2bs"]: np.asarray(fc2_b).astype(BF).reshape(1, D),
    }
    in_maps = []
    for b in range(B):
        m = dict(common)
        m[names["x"]] = np.ascontiguousarray(
            x[b].reshape(TT, P, D).transpose(1, 0, 2))
        m[names["c"]] = np.ascontiguousarray(c[b].reshape(KT, P).T)
        in_maps.append(m)
    return in_maps


def kernel(x, c, ln1_w, ln1_b, ln2_w, ln2_b, ada_w, ada_b,
           qkv_w, qkv_b, proj_w, proj_b, fc1_w, fc1_b, fc2_w, fc2_b,
           _trace=False):
    nc, names = _get_compiled()
    in_maps = _prep_maps(names, x, c, ln1_w, ln1_b, ln2_w, ln2_b,
                         ada_w, ada_b, qkv_w, qkv_b, proj_w, proj_b,
                         fc1_w, fc1_b, fc2_w, fc2_b)
    res = bass_utils.run_bass_kernel_spmd(nc, in_maps, core_ids=list(range(B)),
                                          trace=_trace)
    out = np.stack([res.results[b][names["out"]] for b in range(B)])
    if _trace:
        _CACHE["last_result"] = res
    return out


# revision 34
# speedup vs baseline: 1.0598x; 1.0241x over previous
"""DiT block kernel for 8 Trainium2 NeuronCores.

Sharding: data-parallel over batch (B=8 -> one batch element per core).

Vs the original bf16 baseline:
  - scores / ada / fc1 / fc2 use 512-wide moving operands (baseline used
    256) — wide streams hide the PE weight-load latency.
  - q/k/attn stored fp8(e4m3): halves SBUF + eviction cost; scores and
    attn@V run with fp8 moving operands at bf16 rate; proj streams fp8
    per-column-quantized weights (dequant folded into the gate tile).
  - LN: token-major norm -> PE transpose -> modulate fused into ONE
    tensor_scalar per [128,512] pack at PSUM eviction.
  - fc1 PSUM evicts straight through ACT gelu (bias = per-partition AP).
  - softmax normalize: denominator row is PE-broadcast (ones-matmul) and
    inverted with reciprocal_approx_fast (NOT the 3.7us exact reciprocal).
  - phase-level pipelining: V-GEMM fills attn0's exp-wait; fc2 of chunk
    qc-1 fills attn(qc); gelu/exp never interleave (ACT table thrash).
"""

import sys

sys.path.insert(0, "/opt/trn_rl_repo")

import numpy as np
import ml_dtypes

import concourse.bacc as bacc
import concourse.tile as tile
from concourse import mybir
from concourse import bass_utils
from concourse.masks import make_identity

F32 = mybir.dt.float32
BF16 = mybir.dt.bfloat16
FP8 = mybir.dt.float8e4
BF = ml_dtypes.bfloat16
F8NP = ml_dtypes.float8_e4m3
OP = mybir.AluOpType
AF = mybir.ActivationFunctionType

B = 8
L = 1024
D = 1024
H = 16
HD = 64
FF = 4096
LN_EPS = 1e-5
P = 128
TT = L // P
KT = D // P
FT = FF // P
QC = 512
NQC = L // QC

S_QK = 32.0          # q/k fp8 scale
ESC = 0.125 / (S_QK * S_QK)

_CACHE = {}


def _build():
    nc = bacc.Bacc(None, target_bir_lowering=False)
    names = {}
    with tile.TileContext(nc) as tc, \
            tc.tile_pool(name="dram", bufs=1, space="DRAM") as dram, \
            tc.tile_pool(name="per", bufs=1) as per:

        x_d = dram.tile([P, TT, D], F32, kind="ExternalInput", name="x")
        c_d = dram.tile([P, KT], F32, kind="ExternalInput", name="c")
        lnT_d = dram.tile([P, 4 * KT], F32, kind="ExternalInput", name="lnT")
        adaw_d = dram.tile([P, KT, 6 * D], BF16, kind="ExternalInput",
                           name="ada_w")
        adab_d = dram.tile([1, 6 * D], BF16, kind="ExternalInput",
                           name="ada_b")
        qkvw_d = dram.tile([P, KT, 3 * D], BF16, kind="ExternalInput",
                           name="qkv_w")
        bqk_d = dram.tile([P, 16], F32, kind="ExternalInput", name="bqk")
        vbs_d = dram.tile([1, D], BF16, kind="ExternalInput", name="vbs")
        projw_d = dram.tile([P, KT, D], FP8, kind="ExternalInput",
                            name="proj_w")
        pbs_d = dram.tile([1, D], BF16, kind="ExternalInput", name="pbs")
        dqp_d = dram.tile([1, D], BF16, kind="ExternalInput", name="dqp")
        fc1w_d = dram.tile([P, KT, FF], BF16, kind="ExternalInput",
                           name="fc1_w")
        bf1_d = dram.tile([P, FT], F32, kind="ExternalInput", name="bf1")
        fc2w_d = dram.tile([P, FT, D], BF16, kind="ExternalInput",
                           name="fc2_w")
        f2bs_d = dram.tile([1, D], BF16, kind="ExternalInput", name="f2bs")
        out_d = dram.tile([L, D], F32, kind="ExternalOutput", name="out")
        for t, n in [(x_d, "x"), (c_d, "c"), (lnT_d, "lnT"),
                     (adaw_d, "ada_w"), (adab_d, "ada_b"),
                     (qkvw_d, "qkv_w"), (bqk_d, "bqk"), (vbs_d, "vbs"),
                     (projw_d, "proj_w"), (pbs_d, "pbs"), (dqp_d, "dqp"),
                     (fc1w_d, "fc1_w"), (bf1_d, "bf1"), (fc2w_d, "fc2_w"),
                     (f2bs_d, "f2bs"), (out_d, "out")]:
            names[n] = t.name
        out_v = out_d[:].rearrange("(t p) d -> p t d", p=P)

        x_sb = per.tile([P, TT, D], F32)
        for t in range(TT):
            nc.sync.dma_start(x_sb[:, t, :], x_d[:, t, :])
        ident = per.tile([P, P], BF16)
        make_identity(nc, ident[:])
        onesrow = per.tile([1, P], BF16)
        nc.vector.memset(onesrow[:], 1.0)
        eps_sb = per.tile([P, 1], F32)
        nc.vector.memset(eps_sb[:], LN_EPS)

        lnT = per.tile([P, 4 * KT], F32)
        nc.sync.dma_start(lnT[:], lnT_d[:])
        bqk = per.tile([P, 16], F32)
        nc.sync.dma_start(bqk[:], bqk_d[:])
        vbs = per.tile([1, D], BF16)
        nc.sync.dma_start(vbs[:], vbs_d[:])
        pbs = per.tile([1, D], BF16)
        nc.sync.dma_start(pbs[:], pbs_d[:])
        dqp_row = per.tile([1, D], BF16)
        nc.sync.dma_start(dqp_row[:], dqp_d[:])
        bf1 = per.tile([P, FT], F32)
        nc.sync.dma_start(bf1[:], bf1_d[:])
        f2bs = per.tile([1, D], BF16)
        nc.sync.dma_start(f2bs[:], f2bs_d[:])
        adab = per.tile([1, 6 * D], BF16)
        nc.sync.dma_start(adab[:], adab_d[:])
        projw = per.tile([P, KT, D], FP8)
        for k in range(KT):
            nc.sync.dma_start(projw[:, k, :], projw_d[:, k, :])

        eff1s = per.tile([P, KT], F32)
        eff1h = per.tile([P, KT], F32)
        eff2s = per.tile([P, KT], F32)
        eff2h = per.tile([P, KT], F32)
        g1bc = per.tile([P, D], BF16)
        g2bc = per.tile([P, D], BF16)

        q8 = per.tile([P, KT, L], FP8)
        k8 = per.tile([P, KT, L], FP8)
        v_sb = per.tile([P, TT, H, HD + 1], BF16)
        nc.vector.memset(v_sb[:, :, :, HD:HD + 1], 1.0)
        h2T = per.tile([P, KT, QC], BF16)
        geluT = per.tile([P, FT, QC], BF16)

        _work_cm = tc.tile_pool(name="work", bufs=2)
        work = _work_cm.__enter__()

        def ln_stats(x_aps):
            n = len(x_aps)
            mv = work.tile([P, n, 2], F32, tag=f"mv{n}", name=f"mv{n}")
            for i, x_ap in enumerate(x_aps):
                stats = work.tile([P, 2, 6], F32, tag="stats")
                for sg in range(2):
                    nc.vector.bn_stats(stats[:, sg, :],
                                       x_ap[:, sg * 512:(sg + 1) * 512])
                nc.vector.bn_aggr(mv[:, i, :], stats[:])
            rstd = work.tile([P, n], F32, tag=f"rstd{n}", name=f"rstd{n}")
            nc.scalar.activation(rstd[:], mv[:, :, 1], AF.Sqrt, bias=eps_sb[:])
            nc.vector.reciprocal(rstd[:], rstd[:])
            return mv, rstd

        mv1, rstd1 = ln_stats([x_sb[:, t, :] for t in range(TT)])

        # ---------------- ada (bf16, 512-wide moving weights) -----------
        with nc.named_scope("ada"), \
                tc.tile_pool(name="ada_sb", bufs=1) as asb, \
                tc.tile_pool(name="ada_stream", bufs=2) as ast, \
                tc.tile_pool(name="ps_ada", bufs=2, space="PSUM") as psa, \
                tc.tile_pool(name="ps_t6", bufs=1, space="PSUM") as pst6:
            c_sb = asb.tile([P, KT], F32)
            nc.sync.dma_start(c_sb[:], c_d[:])
            silu_b = asb.tile([P, KT], BF16)
            nc.scalar.activation(silu_b[:], c_sb[:], AF.Silu)
            ssgb = asb.tile([1, 6 * D], BF16, name="ssgb")
            for nch in range(12):
                aw = ast.tile([P, KT, 512], BF16, tag="aw")
                nc.sync.dma_start(aw[:],
                                  adaw_d[:, :, nch * 512:(nch + 1) * 512])
                pa = psa.tile([1, 512], F32, tag="ada")
                nc.tensor.matmul(pa[:], onesrow[:, 0:1],
                                 adab[:, nch * 512:(nch + 1) * 512],
                                 start=True, stop=False)
                for k in range(KT):
                    nc.tensor.matmul(pa[:], silu_b[:, k:k + 1], aw[:, k, :],
                                     start=False, stop=(k == KT - 1),
                                     skip_group_check=True)
                nc.vector.tensor_copy(
                    ssgb[:, nch * 512:(nch + 1) * 512], pa[:])
            # shift/scale rows -> feature-major cols; v' = sh1,sc1,sh2,sc2
            pt6 = pst6.tile([P, 32, 2], BF16)
            for vi, v in enumerate((0, 1, 3, 4)):
                for ch in range(KT):
                    i = ch * 4 + vi
                    nc.tensor.transpose(
                        pt6[:, i, 0:1],
                        ssgb[0:1, v * D + ch * P:v * D + (ch + 1) * P],
                        ident[0:1, 0:1])
            ssgT = asb.tile([P, KT, 4], F32, name="ssgT")
            nc.vector.tensor_copy(ssgT[:], pt6[:, :, 0])
            t0 = asb.tile([P, KT], F32, name="t0")
            t1 = asb.tile([P, KT], F32, name="t1")
            nc.vector.tensor_scalar_add(t0[:], ssgT[:, :, 1], 1.0)
            nc.vector.tensor_mul(eff1s[:], t0[:], lnT[:, 0:KT])
            nc.vector.tensor_mul(t1[:], t0[:], lnT[:, KT:2 * KT])
            nc.vector.tensor_add(eff1h[:], t1[:], ssgT[:, :, 0])
            nc.vector.tensor_scalar_add(t0[:], ssgT[:, :, 3], 1.0)
            nc.vector.tensor_mul(eff2s[:], t0[:], lnT[:, 2 * KT:3 * KT])
            nc.vector.tensor_mul(t1[:], t0[:], lnT[:, 3 * KT:4 * KT])
            nc.vector.tensor_add(eff2h[:], t1[:], ssgT[:, :, 2])
            g1row = asb.tile([1, D], BF16, name="g1row")
            nc.vector.tensor_mul(g1row[:], ssgb[:, 2 * D:3 * D], dqp_row[:])
            nc.gpsimd.partition_broadcast(g1bc[:], g1row[:])
            nc.gpsimd.partition_broadcast(g2bc[:], ssgb[:, 5 * D:6 * D])

        # ---------------- LN -> hT (transpose + fused modulate) ---------
        def build_hT(dst, eff_s, eff_h, mv, rstd, idx0, tg, toff, pstr):
            norms = []
            for s in range(4):
                t = toff + s
                nrm = work.tile([P, D], BF16, tag="nrm", bufs=4)
                i = idx0 + s
                nc.vector.tensor_scalar(out=nrm[:], in0=x_sb[:, t, :],
                                        scalar1=mv[:, i, 0:1],
                                        scalar2=rstd[:, i:i + 1],
                                        op0=OP.subtract, op1=OP.mult)
                norms.append(nrm)
            for k in range(KT):
                pt = pstr.tile([P, 4 * P], BF16, tag="tr")
                for s in range(4):
                    nc.tensor.transpose(pt[:, s * P:(s + 1) * P],
                                        norms[s][:, k * P:(k + 1) * P],
                                        ident[:])
                nc.vector.tensor_scalar(
                    out=dst[:, k, tg * 512:(tg + 1) * 512], in0=pt[:],
                    scalar1=eff_s[:, k:k + 1], scalar2=eff_h[:, k:k + 1],
                    op0=OP.mult, op1=OP.add)

        # ---------------- prologue: h1T + QKV (bf16, 512-wide) ----------
        pro_d = {}
        wv_half = []

        def open_prologue():
            pro_d["pro_cm"] = tc.tile_pool(name="pro", bufs=1)
            pro = pro_d["pro_cm"].__enter__()
            h1T = pro.tile([P, KT, L], BF16, name="h1T")
            with tc.tile_pool(name="ps_tr1", bufs=2, space="PSUM") as pstr1:
                for tg in range(2):
                    build_hT(h1T, eff1s, eff1h, mv1, rstd1, tg * 4, tg,
                             tg * 4, pstr1)
            pro_d["qs_cm"] = tc.tile_pool(name="qkv_stream", bufs=3)
            pro_d["qst"] = pro_d["qs_cm"].__enter__()
            pro_d["h1T"] = h1T

        def qk_chunk(qk, jj):
            fbase = D if qk == 0 else 0
            dst = k8 if qk == 0 else q8
            wj = pro_d["qst"].tile([P, KT, P], BF16, tag="wj", bufs=2,
                                   name="wjt")
            nc.sync.dma_start(
                wj[:], qkvw_d[:, :, fbase + jj * P:fbase + (jj + 1) * P])
            ci = (fbase + jj * P) // P
            for tg in range(2):
                pq = mm_tile(jj * 2 + tg, "pq")
                for k in range(KT):
                    nc.tensor.matmul(pq[:], wj[:, k, :],
                                     pro_d["h1T"][:, k,
                                                  tg * 512:(tg + 1) * 512],
                                     start=(k == 0), stop=(k == KT - 1))
                nc.vector.tensor_scalar(
                    out=dst[:, jj, tg * 512:(tg + 1) * 512], in0=pq[:],
                    scalar1=S_QK, scalar2=bqk[:, ci:ci + 1],
                    op0=OP.mult, op1=OP.add)

        def v_chunk(tt, fh):
            pv = mm_tile(tt * 2 + fh, "pv")
            nc.tensor.matmul(pv[:], onesrow[:, 0:P],
                             vbs[:, fh * 512:(fh + 1) * 512],
                             start=True, stop=False)
            for k in range(KT):
                nc.tensor.matmul(pv[:],
                                 pro_d["h1T"][:, k, tt * P:(tt + 1) * P],
                                 wv_half[fh][:, k, :],
                                 start=False, stop=(k == KT - 1),
                                 skip_group_check=True)
            nc.vector.tensor_copy(
                v_sb[:, tt, fh * 8:(fh + 1) * 8, 0:HD], pv[:])

        def load_wv(fh):
            wv = pro_d["qst"].tile([P, KT, 512], BF16, tag="wv", bufs=1,
                          name=f"wv{fh}")
            nc.sync.dma_start(wv[:],
                              qkvw_d[:, :, 2 * D + fh * 512:
                                     2 * D + (fh + 1) * 512])
            wv_half.append(wv)

        # ---------------- attention + MLP, phase-pipelined --------------
        fsd = {}

        def fs_tile(*a, **k):
            return fsd["fs"].tile(*a, **k)

        with tc.tile_pool(name="attn", bufs=2) as ap, \
                tc.tile_pool(name="aTp", bufs=1) as aTp, \
                tc.tile_pool(name="ps_s", bufs=2, space="PSUM") as pss, \
                tc.tile_pool(name="ps_av", bufs=1, space="PSUM") as psav, \
                tc.tile_pool(name="ps_m", bufs=1, space="PSUM") as psm:

            def mm_tile(i, nm):
                return psm.tile([P, 512], F32, tag=f"f2_{i % 2}", bufs=1,
                                name=nm)

            open_prologue()
            with nc.named_scope("qkv"):
                for jj in range(KT):
                    qk_chunk(0, jj)      # K
                for jj in range(KT):
                    qk_chunk(1, jj)      # Q
                for fh in range(2):
                    load_wv(fh)
                    for tt in range(TT):
                        v_chunk(tt, fh)

            def scores_exp(qc, j):
                q0 = qc * QC
                attA = ap.tile([P, TT, QC], FP8, tag="attA")
                attB = ap.tile([P, TT, QC], FP8, tag="attB")
                for m in range(TT):
                    psA = pss.tile([P, QC], F32, tag="sc", name="psA")
                    psB = pss.tile([P, QC], F32, tag="sc", name="psB")
                    nc.tensor.matmul(psA[:], k8[0:HD, j, m * P:(m + 1) * P],
                                     q8[0:HD, j, q0:q0 + QC],
                                     start=True, stop=True)
                    nc.tensor.matmul(psB[:], k8[HD:P, j, m * P:(m + 1) * P],
                                     q8[HD:P, j, q0:q0 + QC],
                                     start=True, stop=True,
                                     tile_position=(HD, 0))
                    nc.scalar.activation(attA[:, m, :], psA[:], AF.Exp,
                                         scale=ESC)
                    nc.scalar.activation(attB[:, m, :], psB[:], AF.Exp,
                                         scale=ESC)
                return attA, attB

            def av_norm(j, attA, attB, aT):
                for hh, att in ((0, attA), (1, attB)):
                    h = 2 * j + hh
                    pu = psav.tile([HD + 1, QC], F32, tag="pu")
                    for m in range(TT):
                        nc.tensor.matmul(pu[:], v_sb[:, m, h, :],
                                         att[:, m, :],
                                         start=(m == 0), stop=(m == TT - 1))
                    drow = work.tile([1, QC], BF16, tag="drow", bufs=1)
                    nc.vector.tensor_copy(drow[:], pu[HD:HD + 1, :])
                    rb = psav.tile([HD, QC], F32, tag="rb")
                    nc.tensor.matmul(rb[:], onesrow[:, 0:HD], drow[:],
                                     start=True, stop=True)
                    rec = work.tile([HD, QC], F32, tag="rec", bufs=1)
                    nc.vector.reciprocal_approx_fast(rec[:], rb[:])
                    nc.vector.tensor_tensor(
                        out=aT[hh * 64:(hh + 1) * 64, j, :],
                        in0=pu[0:HD, :], in1=rec[:], op=OP.mult)

            def proj_block(qc, aT, s):
                t_global = qc * 4 + s
                for fh in range(2):
                    pp = mm_tile(s * 2 + fh, "pp")
                    nc.tensor.matmul(pp[:], onesrow[:, 0:P],
                                     pbs[:, fh * 512:(fh + 1) * 512],
                                     start=True, stop=False)
                    for k in range(KT):
                        nc.tensor.matmul(
                            pp[:], aT[:, k, s * P:(s + 1) * P],
                            projw[:, k, fh * 512:(fh + 1) * 512],
                            start=False, stop=(k == KT - 1),
                            skip_group_check=True)
                    tmp = work.tile([P, 512], BF16, tag="ptmp", bufs=1)
                    nc.vector.tensor_tensor(
                        out=tmp[:], in0=pp[:],
                        in1=g1bc[:, fh * 512:(fh + 1) * 512], op=OP.mult)
                    nc.vector.tensor_add(
                        x_sb[:, t_global, fh * 512:(fh + 1) * 512],
                        x_sb[:, t_global, fh * 512:(fh + 1) * 512],
                        tmp[:])

            def fc1_gelu(qc):
                for fc in range(FT):
                    w1 = fs_tile([P, KT, P], BF16, tag="w1", bufs=3,
                                 name="w1t")
                    nc.sync.dma_start(w1[:],
                                      fc1w_d[:, :, fc * P:(fc + 1) * P])
                    pf = mm_tile(fc, "pf")
                    for k in range(KT):
                        nc.tensor.matmul(pf[:], w1[:, k, :], h2T[:, k, :],
                                         start=(k == 0), stop=(k == KT - 1))
                    nc.scalar.activation(geluT[:, fc, :], pf[:],
                                         AF.Gelu_apprx_tanh,
                                         bias=bf1[:, fc:fc + 1])

            def fc2_blocks(qc):
                # s-pairs share the streamed w2 chunk; 2 psum banks live
                for sp in range(2):
                    for fp in range(2):
                        def blk(sp=sp, fp=fp, qc=qc):
                            ps2 = [mm_tile(0, "pf2a"), mm_tile(1, "pf2b")]
                            for s in range(2):
                                nc.tensor.matmul(
                                    ps2[s][:], onesrow[:, 0:P],
                                    f2bs[:, fp * 512:(fp + 1) * 512],
                                    start=True, stop=False)
                            for ft in range(FT):
                                w2 = fs_tile([P, 512], BF16, tag="w2",
                                             bufs=3, name="w2t")
                                nc.sync.dma_start(
                                    w2[:], fc2w_d[:, ft, fp * 512:
                                                  (fp + 1) * 512])
                                for s in range(2):
                                    tok = sp * 2 + s
                                    nc.tensor.matmul(
                                        ps2[s][:],
                                        geluT[:, ft,
                                              tok * P:(tok + 1) * P],
                                        w2[:],
                                        start=False, stop=(ft == FT - 1),
                                        skip_group_check=True)
                            for s in range(2):
                                t_global = qc * 4 + sp * 2 + s
                                tmp = work.tile([P, 512], BF16, tag="ftmp",
                                                bufs=1)
                                nc.vector.tensor_tensor(
                                    out=tmp[:], in0=ps2[s][:],
                                    in1=g2bc[:, fp * 512:(fp + 1) * 512],
                                    op=OP.mult)
                                oth = work.tile([P, 512], F32, tag="ot",
                                                bufs=2,
                                                name=f"ot{qc}{fp}{sp}{s}")
                                nc.vector.tensor_add(
                                    oth[:],
                                    x_sb[:, t_global,
                                         fp * 512:(fp + 1) * 512],
                                    tmp[:])
                                nc.sync.dma_start(
                                    out_v[:, t_global,
                                          fp * 512:(fp + 1) * 512],
                                    oth[:])
                        yield blk

            closed = {}

            def run_qc(qc, pending):
                pend_i = 0
                with nc.named_scope(f"attn{qc}"):
                    aT = aTp.tile([P, KT, QC], BF16, tag="aT",
                                  name=f"aT_{qc}")
                    atts = []
                    for j in range(KT):
                        atts.append(scores_exp(qc, j))
                        for _ in range(2):
                            if pend_i < len(pending):
                                pending[pend_i]()
                                pend_i += 1
                        if j >= 1:
                            av_norm(j - 1, *atts[j - 1], aT)
                    av_norm(KT - 1, *atts[KT - 1], aT)
                if qc == 0 and not closed:
                    # h1T / V weights are dead: free prologue SBUF
                    pro_d["qs_cm"].__exit__(None, None, None)
                    pro_d["pro_cm"].__exit__(None, None, None)
                    fsd["cm"] = tc.tile_pool(name="fc_stream", bufs=2)
                    fsd["fs"] = fsd["cm"].__enter__()
                    closed["done"] = True
                with nc.named_scope(f"proj{qc}"):
                    for s in range(4):
                        proj_block(qc, aT, s)
                with nc.named_scope(f"ln2_{qc}"):
                    mv2, rstd2 = ln_stats(
                        [x_sb[:, qc * 4 + s, :] for s in range(4)])
                    with tc.tile_pool(name="ps_tr2", bufs=2,
                                      space="PSUM") as pstr2:
                        build_hT(h2T, eff2s, eff2h, mv2, rstd2, 0, 0,
                                 qc * 4, pstr2)
                with nc.named_scope(f"fc1_{qc}"):
                    fc1_gelu(qc)
                    while pend_i < len(pending):
                        pending[pend_i]()
                        pend_i += 1
                return list(fc2_blocks(qc))

            pending = run_qc(0, [])
            pending = run_qc(1, pending)
            with nc.named_scope("mlp_tail"):
                for blk in pending:
                    blk()
            fsd["cm"].__exit__(None, None, None)

        _work_cm.__exit__(None, None, None)

    nc.compile()
    return nc, names


def _get_compiled():
    if "nc" not in _CACHE:
        _CACHE["nc"], _CACHE["names"] = _build()
    return _CACHE["nc"], _CACHE["names"]


def _q8col(w):
    w = np.asarray(w, np.float32)
    am = np.abs(w).max(axis=0, keepdims=True)
    s = np.where(am > 0, 224.0 / np.maximum(am, 1e-30), 1.0)
    w8 = np.clip(w * s, -240, 240).astype(F8NP)
    return w8, s[0]


def _pmajor(w):
    w = np.asarray(w)
    kp, n = w.shape
    return np.ascontiguousarray(w.reshape(kp // P, P, n).transpose(1, 0, 2))


def _prep_maps(names, x, c, ln1_w, ln1_b, ln2_w, ln2_b, ada_w, ada_b,
               qkv_w, qkv_b, proj_w, proj_b, fc1_w, fc1_b, fc2_w, fc2_b):
    x = np.asarray(x, np.float32)
    c = np.asarray(c, np.float32)
    proj8, s_proj = _q8col(proj_w)
    qkv_b = np.asarray(qkv_b, np.float32)

    def tcols(v):
        return np.asarray(v, np.float32).reshape(KT, P).T
    lnT = np.concatenate([tcols(ln1_w), tcols(ln1_b),
                          tcols(ln2_w), tcols(ln2_b)], axis=1)

    common = {
        names["lnT"]: np.ascontiguousarray(lnT, np.float32),
        names["ada_w"]: _pmajor(ada_w).astype(BF),
        names["ada_b"]: np.asarray(ada_b).astype(BF).reshape(1, -1),
        names["qkv_w"]: _pmajor(qkv_w).astype(BF),
        names["bqk"]: np.ascontiguousarray(
            (qkv_b[0:2 * D] * S_QK).reshape(16, P).T.astype(np.float32)),
        names["vbs"]: qkv_b[2 * D:].astype(BF).reshape(1, D),
        names["proj_w"]: _pmajor(proj8),
        names["pbs"]: (np.asarray(proj_b, np.float32) * s_proj)
        .astype(BF).reshape(1, D),
        names["dqp"]: (1.0 / s_proj).astype(BF).reshape(1, D),
        names["fc1_w"]: _pmajor(fc1_w).astype(BF),
        names["bf1"]: np.ascontiguousarray(
            np.asarray(fc1_b, np.float32).reshape(FT, P).T),
        names["fc2_w"]: _pmajor(fc2_w).astype(BF),
        names["f2bs"]: np.asarray(fc2_b).astype(BF).reshape(1, D),
    }
    in_maps = []
    for b in range(B):
        m = dict(common)
        m[names["x"]] = np.ascontiguousarray(
            x[b].reshape(TT, P, D).transpose(1, 0, 2))
        m[names["c"]] = np.ascontiguousarray(c[b].reshape(KT, P).T)
        in_maps.append(m)
    return in_maps


def kernel(x, c, ln1_w, ln1_b, ln2_w, ln2_b, ada_w, ada_b,
           qkv_w, qkv_b, proj_w, proj_b, fc1_w, fc1_b, fc2_w, fc2_b,
           _trace=False):
    nc, names = _get_compiled()
    in_maps = _prep_maps(names, x, c, ln1_w, ln1_b, ln2_w, ln2_b,
                         ada_w, ada_b, qkv_w, qkv_b, proj_w, proj_b,
                         fc1_w, fc1_b, fc2_w, fc2_b)
    res = bass_utils.run_bass_kernel_spmd(nc, in_maps, core_ids=list(range(B)),
                                          trace=_trace)
    out = np.stack([res.results[b][names["out"]] for b in range(B)])
    if _trace:
        _CACHE["last_result"] = res
    return out


# revision 36
# speedup vs baseline: 1.1147x; 1.0518x over previous
"""DiT block kernel for 8 Trainium2 NeuronCores.

Sharding: data-parallel over batch (B=8 -> one batch element per core).

Vs the original bf16 baseline:
  - scores / ada / fc1 / fc2 use 512-wide moving operands (baseline used
    256) — wide streams hide the PE weight-load latency.
  - q/k/attn stored fp8(e4m3): halves SBUF + eviction cost; scores and
    attn@V run with fp8 moving operands at bf16 rate; proj streams fp8
    per-column-quantized weights (dequant folded into the gate tile).
  - LN: token-major norm -> PE transpose -> modulate fused into ONE
    tensor_scalar per [128,512] pack at PSUM eviction.
  - fc1 PSUM evicts straight through ACT gelu (bias = per-partition AP).
  - softmax normalize: denominator row is PE-broadcast (ones-matmul) and
    inverted with reciprocal_approx_fast (NOT the 3.7us exact reciprocal).
  - phase-level pipelining: V-GEMM fills attn0's exp-wait; fc2 of chunk
    qc-1 fills attn(qc); gelu/exp never interleave (ACT table thrash).
"""

import sys

sys.path.insert(0, "/opt/trn_rl_repo")

import numpy as np
import ml_dtypes

import concourse.bacc as bacc
import concourse.tile as tile
from concourse import mybir
from concourse import bass_utils
from concourse.masks import make_identity

F32 = mybir.dt.float32
BF16 = mybir.dt.bfloat16
FP8 = mybir.dt.float8e4
BF = ml_dtypes.bfloat16
F8NP = ml_dtypes.float8_e4m3
OP = mybir.AluOpType
AF = mybir.ActivationFunctionType

B = 8
L = 1024
D = 1024
H = 16
HD = 64
FF = 4096
LN_EPS = 1e-5
P = 128
TT = L // P
KT = D // P
FT = FF // P
QC = 512
NQC = L // QC

S_QK = 32.0          # q/k fp8 scale
ESC = 0.125 / (S_QK * S_QK)

_CACHE = {}


def _build():
    nc = bacc.Bacc(None, target_bir_lowering=False)
    names = {}
    with tile.TileContext(nc) as tc, \
            tc.tile_pool(name="dram", bufs=1, space="DRAM") as dram, \
            tc.tile_pool(name="per", bufs=1) as per:

        x_d = dram.tile([P, TT, D], F32, kind="ExternalInput", name="x")
        c_d = dram.tile([P, KT], F32, kind="ExternalInput", name="c")
        lnT_d = dram.tile([P, 4 * KT], F32, kind="ExternalInput", name="lnT")
        adaw_d = dram.tile([P, KT, 6 * D], BF16, kind="ExternalInput",
                           name="ada_w")
        adab_d = dram.tile([1, 6 * D], BF16, kind="ExternalInput",
                           name="ada_b")
        qkvw_d = dram.tile([P, KT, 3 * D], BF16, kind="ExternalInput",
                           name="qkv_w")
        bqk_d = dram.tile([P, 16], F32, kind="ExternalInput", name="bqk")
        vbs_d = dram.tile([1, D], BF16, kind="ExternalInput", name="vbs")
        projw_d = dram.tile([P, KT, D], FP8, kind="ExternalInput",
                            name="proj_w")
        pbs_d = dram.tile([1, D], BF16, kind="ExternalInput", name="pbs")
        dqp_d = dram.tile([1, D], BF16, kind="ExternalInput", name="dqp")
        fc1w_d = dram.tile([P, KT, FF], BF16, kind="ExternalInput",
                           name="fc1_w")
        bf1_d = dram.tile([1, FF], BF16, kind="ExternalInput", name="bf1")
        fc2w_d = dram.tile([P, FT, D], BF16, kind="ExternalInput",
                           name="fc2_w")
        f2bs_d = dram.tile([1, D], BF16, kind="ExternalInput", name="f2bs")
        out_d = dram.tile([L, D], F32, kind="ExternalOutput", name="out")
        for t, n in [(x_d, "x"), (c_d, "c"), (lnT_d, "lnT"),
                     (adaw_d, "ada_w"), (adab_d, "ada_b"),
                     (qkvw_d, "qkv_w"), (bqk_d, "bqk"), (vbs_d, "vbs"),
                     (projw_d, "proj_w"), (pbs_d, "pbs"), (dqp_d, "dqp"),
                     (fc1w_d, "fc1_w"), (bf1_d, "bf1"), (fc2w_d, "fc2_w"),
                     (f2bs_d, "f2bs"), (out_d, "out")]:
            names[n] = t.name
        out_v = out_d[:].rearrange("(t p) d -> p t d", p=P)

        x_sb = per.tile([P, TT, D], F32)
        for t in range(TT):
            nc.sync.dma_start(x_sb[:, t, :], x_d[:, t, :])
        ident = per.tile([P, P], BF16)
        make_identity(nc, ident[:])
        onesrow = per.tile([1, 512], BF16)
        nc.vector.memset(onesrow[:], 1.0)
        eps_sb = per.tile([P, 1], F32)
        nc.vector.memset(eps_sb[:], LN_EPS)

        lnT = per.tile([P, 4 * KT], F32)
        nc.sync.dma_start(lnT[:], lnT_d[:])
        bqk = per.tile([P, 16], F32)
        nc.sync.dma_start(bqk[:], bqk_d[:])
        vbs = per.tile([1, D], BF16)
        nc.sync.dma_start(vbs[:], vbs_d[:])
        pbs = per.tile([1, D], BF16)
        nc.sync.dma_start(pbs[:], pbs_d[:])
        dqp_row = per.tile([1, D], BF16)
        nc.sync.dma_start(dqp_row[:], dqp_d[:])

        f2bs = per.tile([1, D], BF16)
        nc.sync.dma_start(f2bs[:], f2bs_d[:])
        adab = per.tile([1, 6 * D], BF16)
        nc.sync.dma_start(adab[:], adab_d[:])
        projw = per.tile([P, KT, D], FP8)
        for k in range(KT):
            nc.sync.dma_start(projw[:, k, :], projw_d[:, k, :])

        eff1s = per.tile([P, KT], F32)
        eff1h = per.tile([P, KT], F32)
        eff2s = per.tile([P, KT], F32)
        eff2h = per.tile([P, KT], F32)
        g1bc = per.tile([P, D], BF16)
        g2bc = per.tile([P, D], BF16)

        q8 = per.tile([P, KT, L], FP8)
        k8 = per.tile([P, KT, L], FP8)
        v_sb = per.tile([P, TT, H, HD + 1], BF16)
        nc.vector.memset(v_sb[:, :, :, HD:HD + 1], 1.0)
        h2T = per.tile([P, KT, QC], BF16)
        geluT = per.tile([P, FT, QC], BF16)

        _work_cm = tc.tile_pool(name="work", bufs=2)
        work = _work_cm.__enter__()

        def ln_stats(x_aps):
            n = len(x_aps)
            mv = work.tile([P, n, 2], F32, tag=f"mv{n}", name=f"mv{n}")
            for i, x_ap in enumerate(x_aps):
                stats = work.tile([P, 2, 6], F32, tag="stats")
                for sg in range(2):
                    nc.vector.bn_stats(stats[:, sg, :],
                                       x_ap[:, sg * 512:(sg + 1) * 512])
                nc.vector.bn_aggr(mv[:, i, :], stats[:])
            rstd = work.tile([P, n], F32, tag=f"rstd{n}", name=f"rstd{n}")
            nc.scalar.activation(rstd[:], mv[:, :, 1], AF.Sqrt, bias=eps_sb[:])
            nc.vector.reciprocal(rstd[:], rstd[:])
            return mv, rstd

        mv1, rstd1 = ln_stats([x_sb[:, t, :] for t in range(TT)])

        # ---------------- ada (bf16, 512-wide moving weights) -----------
        with nc.named_scope("ada"), \
                tc.tile_pool(name="ada_sb", bufs=1) as asb, \
                tc.tile_pool(name="ada_stream", bufs=2) as ast, \
                tc.tile_pool(name="ps_ada", bufs=2, space="PSUM") as psa, \
                tc.tile_pool(name="ps_t6", bufs=1, space="PSUM") as pst6:
            c_sb = asb.tile([P, KT], F32)
            nc.sync.dma_start(c_sb[:], c_d[:])
            silu_b = asb.tile([P, KT], BF16)
            nc.scalar.activation(silu_b[:], c_sb[:], AF.Silu)
            ssgb = asb.tile([1, 6 * D], BF16, name="ssgb")
            for nch in range(12):
                aw = ast.tile([P, KT, 512], BF16, tag="aw")
                nc.sync.dma_start(aw[:],
                                  adaw_d[:, :, nch * 512:(nch + 1) * 512])
                pa = psa.tile([1, 512], F32, tag="ada")
                nc.tensor.matmul(pa[:], onesrow[:, 0:1],
                                 adab[:, nch * 512:(nch + 1) * 512],
                                 start=True, stop=False)
                for k in range(KT):
                    nc.tensor.matmul(pa[:], silu_b[:, k:k + 1], aw[:, k, :],
                                     start=False, stop=(k == KT - 1),
                                     skip_group_check=True)
                nc.vector.tensor_copy(
                    ssgb[:, nch * 512:(nch + 1) * 512], pa[:])
            # shift/scale rows -> feature-major cols; v' = sh1,sc1,sh2,sc2
            pt6 = pst6.tile([P, 32, 2], BF16)
            for vi, v in enumerate((0, 1, 3, 4)):
                for ch in range(KT):
                    i = ch * 4 + vi
                    nc.tensor.transpose(
                        pt6[:, i, 0:1],
                        ssgb[0:1, v * D + ch * P:v * D + (ch + 1) * P],
                        ident[0:1, 0:1])
            ssgT = asb.tile([P, KT, 4], F32, name="ssgT")
            nc.vector.tensor_copy(ssgT[:], pt6[:, :, 0])
            t0 = asb.tile([P, KT], F32, name="t0")
            t1 = asb.tile([P, KT], F32, name="t1")
            nc.vector.tensor_scalar_add(t0[:], ssgT[:, :, 1], 1.0)
            nc.vector.tensor_mul(eff1s[:], t0[:], lnT[:, 0:KT])
            nc.vector.tensor_mul(t1[:], t0[:], lnT[:, KT:2 * KT])
            nc.vector.tensor_add(eff1h[:], t1[:], ssgT[:, :, 0])
            nc.vector.tensor_scalar_add(t0[:], ssgT[:, :, 3], 1.0)
            nc.vector.tensor_mul(eff2s[:], t0[:], lnT[:, 2 * KT:3 * KT])
            nc.vector.tensor_mul(t1[:], t0[:], lnT[:, 3 * KT:4 * KT])
            nc.vector.tensor_add(eff2h[:], t1[:], ssgT[:, :, 2])
            g1row = asb.tile([1, D], BF16, name="g1row")
            nc.vector.tensor_mul(g1row[:], ssgb[:, 2 * D:3 * D], dqp_row[:])
            nc.gpsimd.partition_broadcast(g1bc[:], g1row[:])
            nc.gpsimd.partition_broadcast(g2bc[:], ssgb[:, 5 * D:6 * D])

        # ---------------- LN -> hT (transpose + fused modulate) ---------
        def build_hT(dst, eff_s, eff_h, mv, rstd, idx0, tg, toff, pstr):
            norms = []
            for s in range(4):
                t = toff + s
                nrm = work.tile([P, D], BF16, tag="nrm", bufs=4)
                i = idx0 + s
                nc.vector.tensor_scalar(out=nrm[:], in0=x_sb[:, t, :],
                                        scalar1=mv[:, i, 0:1],
                                        scalar2=rstd[:, i:i + 1],
                                        op0=OP.subtract, op1=OP.mult)
                norms.append(nrm)
            for k in range(KT):
                pt = pstr.tile([P, 4 * P], BF16, tag="tr")
                for s in range(4):
                    nc.tensor.transpose(pt[:, s * P:(s + 1) * P],
                                        norms[s][:, k * P:(k + 1) * P],
                                        ident[:])
                nc.vector.tensor_scalar(
                    out=dst[:, k, tg * 512:(tg + 1) * 512], in0=pt[:],
                    scalar1=eff_s[:, k:k + 1], scalar2=eff_h[:, k:k + 1],
                    op0=OP.mult, op1=OP.add)

        # ---------------- prologue: h1T + QKV (bf16, 512-wide) ----------
        pro_d = {}
        wv_half = []

        def open_prologue():
            pro_d["pro_cm"] = tc.tile_pool(name="pro", bufs=1)
            pro = pro_d["pro_cm"].__enter__()
            h1T = pro.tile([P, KT, L], BF16, name="h1T")
            with tc.tile_pool(name="ps_tr1", bufs=2, space="PSUM") as pstr1:
                for tg in range(2):
                    build_hT(h1T, eff1s, eff1h, mv1, rstd1, tg * 4, tg,
                             tg * 4, pstr1)
            pro_d["qs_cm"] = tc.tile_pool(name="qkv_stream", bufs=3)
            pro_d["qst"] = pro_d["qs_cm"].__enter__()
            pro_d["h1T"] = h1T

        def qk_chunk(qk, jj):
            fbase = D if qk == 0 else 0
            dst = k8 if qk == 0 else q8
            wj = pro_d["qst"].tile([P, KT, P], BF16, tag="wj", bufs=2,
                                   name="wjt")
            nc.sync.dma_start(
                wj[:], qkvw_d[:, :, fbase + jj * P:fbase + (jj + 1) * P])
            ci = (fbase + jj * P) // P
            for tg in range(2):
                pq = mm_tile(jj * 2 + tg, "pq")
                for k in range(KT):
                    nc.tensor.matmul(pq[:], wj[:, k, :],
                                     pro_d["h1T"][:, k,
                                                  tg * 512:(tg + 1) * 512],
                                     start=(k == 0), stop=(k == KT - 1))
                nc.vector.tensor_scalar(
                    out=dst[:, jj, tg * 512:(tg + 1) * 512], in0=pq[:],
                    scalar1=S_QK, scalar2=bqk[:, ci:ci + 1],
                    op0=OP.mult, op1=OP.add)

        def v_chunk(tt, fh):
            pv = mm_tile(tt * 2 + fh, "pv")
            nc.tensor.matmul(pv[:], onesrow[:, 0:P],
                             vbs[:, fh * 512:(fh + 1) * 512],
                             start=True, stop=False)
            for k in range(KT):
                nc.tensor.matmul(pv[:],
                                 pro_d["h1T"][:, k, tt * P:(tt + 1) * P],
                                 wv_half[fh][:, k, :],
                                 start=False, stop=(k == KT - 1),
                                 skip_group_check=True)
            nc.vector.tensor_copy(
                v_sb[:, tt, fh * 8:(fh + 1) * 8, 0:HD], pv[:])

        def load_wv(fh):
            wv = pro_d["qst"].tile([P, KT, 512], BF16, tag="wv", bufs=1,
                          name=f"wv{fh}")
            nc.sync.dma_start(wv[:],
                              qkvw_d[:, :, 2 * D + fh * 512:
                                     2 * D + (fh + 1) * 512])
            wv_half.append(wv)

        # ---------------- attention + MLP, phase-pipelined --------------
        fsd = {}

        def fs_tile(*a, **k):
            return fsd["fs"].tile(*a, **k)

        with tc.tile_pool(name="attn", bufs=2) as ap, \
                tc.tile_pool(name="aTp", bufs=1) as aTp, \
                tc.tile_pool(name="ps_s", bufs=2, space="PSUM") as pss, \
                tc.tile_pool(name="ps_av", bufs=1, space="PSUM") as psav, \
                tc.tile_pool(name="ps_m", bufs=1, space="PSUM") as psm:

            def mm_tile(i, nm):
                return psm.tile([P, 512], F32, tag=f"f2_{i % 2}", bufs=1,
                                name=nm)

            open_prologue()
            with nc.named_scope("qkv"):
                for jj in range(KT):
                    qk_chunk(0, jj)      # K
                for jj in range(KT):
                    qk_chunk(1, jj)      # Q
                for fh in range(2):
                    load_wv(fh)
                    for tt in range(TT):
                        v_chunk(tt, fh)

            def scores_exp(qc, j):
                q0 = qc * QC
                attA = ap.tile([P, TT, QC], FP8, tag="attA")
                attB = ap.tile([P, TT, QC], FP8, tag="attB")
                for m in range(TT):
                    psA = pss.tile([P, QC], F32, tag="sc", name="psA")
                    psB = pss.tile([P, QC], F32, tag="sc", name="psB")
                    nc.tensor.matmul(psA[:], k8[0:HD, j, m * P:(m + 1) * P],
                                     q8[0:HD, j, q0:q0 + QC],
                                     start=True, stop=True)
                    nc.tensor.matmul(psB[:], k8[HD:P, j, m * P:(m + 1) * P],
                                     q8[HD:P, j, q0:q0 + QC],
                                     start=True, stop=True,
                                     tile_position=(HD, 0))
                    nc.scalar.activation(attA[:, m, :], psA[:], AF.Exp,
                                         scale=ESC)
                    nc.scalar.activation(attB[:, m, :], psB[:], AF.Exp,
                                         scale=ESC)
                return attA, attB

            def av_norm(j, attA, attB, aT):
                for hh, att in ((0, attA), (1, attB)):
                    h = 2 * j + hh
                    pu = psav.tile([HD + 1, QC], F32, tag="pu")
                    for m in range(TT):
                        nc.tensor.matmul(pu[:], v_sb[:, m, h, :],
                                         att[:, m, :],
                                         start=(m == 0), stop=(m == TT - 1))
                    drow = work.tile([1, QC], BF16, tag="drow", bufs=1)
                    nc.vector.tensor_copy(drow[:], pu[HD:HD + 1, :])
                    rb = psav.tile([HD, QC], F32, tag="rb")
                    nc.tensor.matmul(rb[:], onesrow[:, 0:HD], drow[:],
                                     start=True, stop=True)
                    rec = work.tile([HD, QC], F32, tag="rec", bufs=1)
                    nc.vector.reciprocal_approx_fast(rec[:], rb[:])
                    nc.vector.tensor_tensor(
                        out=aT[hh * 64:(hh + 1) * 64, j, :],
                        in0=pu[0:HD, :], in1=rec[:], op=OP.mult)

            def proj_block(qc, aT, s):
                t_global = qc * 4 + s
                for fh in range(2):
                    pp = mm_tile(s * 2 + fh, "pp")
                    nc.tensor.matmul(pp[:], onesrow[:, 0:P],
                                     pbs[:, fh * 512:(fh + 1) * 512],
                                     start=True, stop=False)
                    for k in range(KT):
                        nc.tensor.matmul(
                            pp[:], aT[:, k, s * P:(s + 1) * P],
                            projw[:, k, fh * 512:(fh + 1) * 512],
                            start=False, stop=(k == KT - 1),
                            skip_group_check=True)
                    tmp = work.tile([P, 512], BF16, tag="ptmp", bufs=1)
                    nc.vector.tensor_tensor(
                        out=tmp[:], in0=pp[:],
                        in1=g1bc[:, fh * 512:(fh + 1) * 512], op=OP.mult)
                    nc.vector.tensor_add(
                        x_sb[:, t_global, fh * 512:(fh + 1) * 512],
                        x_sb[:, t_global, fh * 512:(fh + 1) * 512],
                        tmp[:])

            def fc1_gelu(qc):
                if "bf1" not in fsd:
                    bf1 = fs_tile([1, FF], BF16, tag="bf1", bufs=1,
                                  name="bf1r")
                    nc.sync.dma_start(bf1[:], bf1_d[:])
                    fsd["bf1"] = bf1
                bf1 = fsd["bf1"]
                for fc in range(FT):
                    w1 = fs_tile([P, KT, P], BF16, tag="w1", bufs=3,
                                 name="w1t")
                    nc.sync.dma_start(w1[:],
                                      fc1w_d[:, :, fc * P:(fc + 1) * P])
                    pf = mm_tile(fc, "pf")
                    nc.tensor.matmul(pf[:], bf1[:, fc * P:(fc + 1) * P],
                                     onesrow[:, 0:512],
                                     start=True, stop=False)
                    for k in range(KT):
                        nc.tensor.matmul(pf[:], w1[:, k, :], h2T[:, k, :],
                                         start=False, stop=(k == KT - 1),
                                         skip_group_check=True)
                    nc.scalar.activation(geluT[:, fc, :], pf[:],
                                         AF.Gelu_apprx_tanh)

            def fc2_blocks(qc):
                # s-pairs share the streamed w2 chunk; 2 psum banks live
                for sp in range(2):
                    for fp in range(2):
                        def blk(sp=sp, fp=fp, qc=qc):
                            ps2 = [mm_tile(0, "pf2a"), mm_tile(1, "pf2b")]
                            for s in range(2):
                                nc.tensor.matmul(
                                    ps2[s][:], onesrow[:, 0:P],
                                    f2bs[:, fp * 512:(fp + 1) * 512],
                                    start=True, stop=False)
                            for ft in range(FT):
                                w2 = fs_tile([P, 512], BF16, tag="w2",
                                             bufs=3, name="w2t")
                                nc.sync.dma_start(
                                    w2[:], fc2w_d[:, ft, fp * 512:
                                                  (fp + 1) * 512])
                                for s in range(2):
                                    tok = sp * 2 + s
                                    nc.tensor.matmul(
                                        ps2[s][:],
                                        geluT[:, ft,
                                              tok * P:(tok + 1) * P],
                                        w2[:],
                                        start=False, stop=(ft == FT - 1),
                                        skip_group_check=True)
                            for s in range(2):
                                t_global = qc * 4 + sp * 2 + s
                                tmp = work.tile([P, 512], BF16, tag="ftmp",
                                                bufs=1)
                                nc.vector.tensor_tensor(
                                    out=tmp[:], in0=ps2[s][:],
                                    in1=g2bc[:, fp * 512:(fp + 1) * 512],
                                    op=OP.mult)
                                oth = work.tile([P, 512], F32, tag="ot",
                                                bufs=2,
                                                name=f"ot{qc}{fp}{sp}{s}")
                                nc.vector.tensor_add(
                                    oth[:],
                                    x_sb[:, t_global,
                                         fp * 512:(fp + 1) * 512],
                                    tmp[:])
                                nc.sync.dma_start(
                                    out_v[:, t_global,
                                          fp * 512:(fp + 1) * 512],
                                    oth[:])
                        yield blk

            closed = {}

            def run_qc(qc, pending):
                pend_i = 0
                with nc.named_scope(f"attn{qc}"):
                    aT = aTp.tile([P, KT, QC], BF16, tag="aT",
                                  name=f"aT_{qc}")
                    atts = []
                    for j in range(KT):
                        atts.append(scores_exp(qc, j))
                        for _ in range(2):
                            if pend_i < len(pending):
                                pending[pend_i]()
                                pend_i += 1
                        if j >= 1:
                            av_norm(j - 1, *atts[j - 1], aT)
                    av_norm(KT - 1, *atts[KT - 1], aT)
                if qc == 0 and not closed:
                    # h1T / V weights are dead: free prologue SBUF
                    pro_d["qs_cm"].__exit__(None, None, None)
                    pro_d["pro_cm"].__exit__(None, None, None)
                    fsd["cm"] = tc.tile_pool(name="fc_stream", bufs=2)
                    fsd["fs"] = fsd["cm"].__enter__()
                    closed["done"] = True
                with nc.named_scope(f"proj{qc}"):
                    for s in range(4):
                        proj_block(qc, aT, s)
                with nc.named_scope(f"ln2_{qc}"):
                    mv2, rstd2 = ln_stats(
                        [x_sb[:, qc * 4 + s, :] for s in range(4)])
                    with tc.tile_pool(name="ps_tr2", bufs=2,
                                      space="PSUM") as pstr2:
                        build_hT(h2T, eff2s, eff2h, mv2, rstd2, 0, 0,
                                 qc * 4, pstr2)
                with nc.named_scope(f"fc1_{qc}"):
                    fc1_gelu(qc)
                    while pend_i < len(pending):
                        pending[pend_i]()
                        pend_i += 1
                return list(fc2_blocks(qc))

            pending = run_qc(0, [])
            pending = run_qc(1, pending)
            with nc.named_scope("mlp_tail"):
                for blk in pending:
                    blk()
            fsd["cm"].__exit__(None, None, None)

        _work_cm.__exit__(None, None, None)

    nc.compile()
    return nc, names


def _get_compiled():
    if "nc" not in _CACHE:
        _CACHE["nc"], _CACHE["names"] = _build()
    return _CACHE["nc"], _CACHE["names"]


def _q8col(w):
    w = np.asarray(w, np.float32)
    am = np.abs(w).max(axis=0, keepdims=True)
    s = np.where(am > 0, 224.0 / np.maximum(am, 1e-30), 1.0)
    w8 = np.clip(w * s, -240, 240).astype(F8NP)
    return w8, s[0]


def _pmajor(w):
    w = np.asarray(w)
    kp, n = w.shape
    return np.ascontiguousarray(w.reshape(kp // P, P, n).transpose(1, 0, 2))


def _prep_maps(names, x, c, ln1_w, ln1_b, ln2_w, ln2_b, ada_w, ada_b,
               qkv_w, qkv_b, proj_w, proj_b, fc1_w, fc1_b, fc2_w, fc2_b):
    x = np.asarray(x, np.float32)
    c = np.asarray(c, np.float32)
    proj8, s_proj = _q8col(proj_w)
    qkv_b = np.asarray(qkv_b, np.float32)

    def tcols(v):
        return np.asarray(v, np.float32).reshape(KT, P).T
    lnT = np.concatenate([tcols(ln1_w), tcols(ln1_b),
                          tcols(ln2_w), tcols(ln2_b)], axis=1)

    common = {
        names["lnT"]: np.ascontiguousarray(lnT, np.float32),
        names["ada_w"]: _pmajor(ada_w).astype(BF),
        names["ada_b"]: np.asarray(ada_b).astype(BF).reshape(1, -1),
        names["qkv_w"]: _pmajor(qkv_w).astype(BF),
        names["bqk"]: np.ascontiguousarray(
            (qkv_b[0:2 * D] * S_QK).reshape(16, P).T.astype(np.float32)),
        names["vbs"]: qkv_b[2 * D:].astype(BF).reshape(1, D),
        names["proj_w"]: _pmajor(proj8),
        names["pbs"]: (np.asarray(proj_b, np.float32) * s_proj)
        .astype(BF).reshape(1, D),
        names["dqp"]: (1.0 / s_proj).astype(BF).reshape(1, D),
        names["fc1_w"]: _pmajor(fc1_w).astype(BF),
        names["bf1"]: np.asarray(fc1_b).astype(BF).reshape(1, FF),
        names["fc2_w"]: _pmajor(fc2_w).astype(BF),
        names["f2bs"]: np.asarray(fc2_b).astype(BF).reshape(1, D),
    }
    in_maps = []
    for b in range(B):
        m = dict(common)
        m[names["x"]] = np.ascontiguousarray(
            x[b].reshape(TT, P, D).transpose(1, 0, 2))
        m[names["c"]] = np.ascontiguousarray(c[b].reshape(KT, P).T)
        in_maps.append(m)
    return in_maps


def kernel(x, c, ln1_w, ln1_b, ln2_w, ln2_b, ada_w, ada_b,
           qkv_w, qkv_b, proj_w, proj_b, fc1_w, fc1_b, fc2_w, fc2_b,
           _trace=False):
    nc, names = _get_compiled()
    in_maps = _prep_maps(names, x, c, ln1_w, ln1_b, ln2_w, ln2_b,
                         ada_w, ada_b, qkv_w, qkv_b, proj_w, proj_b,
                         fc1_w, fc1_b, fc2_w, fc2_b)
    res = bass_utils.run_bass_kernel_spmd(nc, in_maps, core_ids=list(range(B)),
                                          trace=_trace)
    out = np.stack([res.results[b][names["out"]] for b in range(B)])
    if _trace:
        _CACHE["last_result"] = res
    return out
